# revision 2
# baseline (speedup 1.0000x reference)
"""AdaptiveResonateAndFireNeuron Trainium2 kernel v2 (8 cores, data-parallel batch).

Per batch row b:
  proj[t] = x[b,t,:] @ W.T + bias
  u       = 0.9*mem + proj[t] - 0.1*sum_o(mem)
  spk     = (u >= thr);  mem' = u - spk*thr;  thr' = 0.9*thr + 0.1*spk

v2 structure (vs v1 baseline):
- phase 1 (proj) matmuls in float32r (1 cyc/row at 512 cols vs 4 for fp32);
  PSUM->SBUF copy + bias moved from DVE to the Activation engine.
- phase 2: u is accumulated ENTIRELY in PSUM by 6 matmuls per step per chain
  (identity@proj + 4x (-0.1*ones)@mem [o-block reduce] + 0.9*identity@mem).
  A single custom DVE op (ARF_PAIRFIRE, subdim pages) then reads PSUM once and
  writes BOTH outputs: page0 = spike (straight into the DMA staging buffer),
  page1 = mem' (into the mem trajectory ring).  Threshold state is kept scaled
  (sigma = 10*thr) and updated on the GpSimd/Pool engine with ONE
  scalar_tensor_tensor: sigma' = 0.9*sigma + spk.
- two independent chains (2 batches each) interleave so the PE->DVE->PE
  latency of one chain hides under the other.

Core layout: partition p = o_in (o % 128); per-step col c = b4*4 + k where
b4 = local batch (0..3), k = o_blk (o // 128); chain q = b4 // 2.
"""

import numpy as np

B, S, I, O = 32, 1024, 512, 512

SIGMA_ENGINE = "gpsimd"          # "gpsimd" (Pool) or "vector" (DVE)


def _SIGMA_ENG(nc):
    return getattr(nc, SIGMA_ENGINE)
NB = B // 8          # batches per core
DECAY = 0.9
INHIB = 0.1

_CACHE = {}


def _register_dve_ops():
    import concourse.dve_ops as dve_ops
    from concourse.dve_spec import Spec, Src0, Src1, C1, select, minn, SubIdx
    from concourse.dve_table_gen import dve_ver_for

    if "ARF_PAIRFIRE" in dve_ops._SUB_OPCODE_FOR_NAME:
        from concourse.dve_ops import OPS
        by_name = {op.name: op for op in OPS}
        return by_name["ARF_PAIRFIRE"]

    # in0 = u (PSUM, page-broadcast), in1 = sigma = 10*thr (page-broadcast),
    # s1 (C1) = 0.1.  h = 0.1*sigma = thr.
    # page 0 (SubIdx falsy) -> spike = (u >= thr)
    # page 1                -> mem'  = min(u, u - thr)   (valid: thr > 0)
    h = Src1 * C1
    body = select(SubIdx, minn(Src0, Src0 - h), Src0 >= h)

    def ref(in0, in1, s0, s1, imm2):
        u = in0.astype(np.float32)
        thr = (in1 * np.float32(s1)).astype(np.float32)
        spk = (u >= thr).astype(np.float32)
        memf = np.minimum(u, (u - thr).astype(np.float32))
        pages = np.arange(in0.shape[1])[None, :, None]
        return np.where(pages == 0, spk, memf).astype(np.float32)

    spec = Spec(body=body, reference=ref)
    op = dve_ops.DveOp("ARF_PAIRFIRE", spec, subdim=True, uops_sha={})
    dve_ops.OPS.append(op)
    dve_ops.CUSTOM_DVE_SPECS["ARF_PAIRFIRE"] = spec
    dve_ops._SUB_OPCODE_FOR_NAME["ARF_PAIRFIRE"] = (
        dve_ops._CUSTOM_DVE_ROW_BASE + len(dve_ops.OPS) - 1
    )
    ver = dve_ver_for("TRN2")
    try:
        op.compile(ver)
    except ValueError as e:
        import re

        m = re.search(r":\s*([0-9a-f]{8,})\s*≠", str(e))
        if m is None:
            raise
        op.uops_sha[ver] = m.group(1)
        dve_ops._COMPILE_CACHE.pop(("ARF_PAIRFIRE", ver), None)
        op.compile(ver)
    return op


def _build(n_steps):
    import concourse.bass as bass
    from concourse import bacc
    import concourse.mybir as mybir
    from concourse import tile

    PAIRFIRE = _register_dve_ops()

    f32 = mybir.dt.float32
    f32r = mybir.dt.float32r
    Alu = mybir.AluOpType
    Act = mybir.ActivationFunctionType

    nc = bacc.Bacc()
    # xt[i, t*4+b4] = x[b4, t, i]  (host pre-transposed, per core)
    xt = nc.declare_dram_parameter("xt", [I, NB * n_steps], f32, isOutput=False)
    # wt[i, o] = W[o, i]
    wt = nc.declare_dram_parameter("wt", [I, O], f32, isOutput=False)
    # br[o_in, k] = bias[k*128 + o_in]
    br = nc.declare_dram_parameter("br", [128, 4], f32, isOutput=False)
    # eyes[:, 0:128] = I, eyes[:, 128:256] = 0.9*I
    eyes = nc.declare_dram_parameter("eyes", [128, 256], f32, isOutput=False)
    # out[o_in, q*(n_steps*8) + t*8 + b2*4 + k] = spikes[q*2+b2, t, k*128+o_in]
    out_d = nc.declare_dram_parameter("out", [128, n_steps * 16], f32, isOutput=True)

    KB = I // 128            # 4 contraction blocks
    OB = O // 128            # 4 output blocks
    TCH = min(128, n_steps)  # timesteps per chunk
    assert n_steps % TCH == 0
    NCH = n_steps // TCH

    # per-chain mega-tile column layout: [OUT0 | MB0 | OUT1 | MB1]
    # (separate tile per chain so hazard intervals never cross chains)
    OUTW = TCH * 8
    MBW = (TCH + 1) * 8
    OUTC = [0, OUTW + MBW]
    MBC = [OUTW, 2 * OUTW + MBW]
    MEGA_W = 2 * (OUTW + MBW)
    DPAGE = MBC[0] - OUTC[0] + 8      # +8: mem slot t+1 vs spike slot t

    with tile.TileContext(nc) as tc:
        with (
            tc.tile_pool(name="const", bufs=1) as constp,
            tc.tile_pool(name="state", bufs=1) as statep,
            tc.tile_pool(name="big", bufs=1) as bigp,
            tc.tile_pool(name="xin", bufs=8) as xinp,
            tc.tile_pool(name="ps1", bufs=4, space=bass.MemorySpace.PSUM) as ps1p,
            tc.tile_pool(name="ps2", bufs=4, space=bass.MemorySpace.PSUM) as ps2p,
        ):
            # ---- constants (DMA + ACT proxy copy so PE waits on one sem) ----
            w_raw = constp.tile([128, KB * O], f32, tag="wraw")
            w_sb = constp.tile([128, KB * O], f32, tag="w")
            br_raw = constp.tile([128, 4], f32, tag="brraw")
            br_sb = constp.tile([128, 4], f32, tag="br")
            ey_raw = constp.tile([128, 256], f32, tag="eyraw")
            ey_sb = constp.tile([128, 256], f32, tag="ey")
            ones_sb = constp.tile([128, 128], f32, tag="ones")
            for k in range(KB):
                nc.gpsimd.dma_start(w_raw[:, k * O:(k + 1) * O], wt[k * 128:(k + 1) * 128, :])
                nc.scalar.copy(w_sb[:, k * O:(k + 1) * O].bitcast(f32r), w_raw[:, k * O:(k + 1) * O])
            nc.gpsimd.dma_start(br_raw[:], br[:])
            nc.scalar.copy(br_sb[:], br_raw[:])
            nc.gpsimd.dma_start(ey_raw[:], eyes[:])
            nc.scalar.copy(ey_sb[:], ey_raw[:])
            nc.vector.memset(ones_sb[:], -INHIB)
            eye1 = ey_sb[:, 0:128]
            eye9 = ey_sb[:, 128:256]

            # ---- persistent buffers ----
            proj = bigp.tile([128, n_steps * 16], f32, tag="proj")
            megaA = bigp.tile([128, MEGA_W], f32, tag="megaA")
            megaB = bigp.tile([128, MEGA_W], f32, tag="megaB")
            mega = [megaA, megaB]
            proj4 = proj[:].rearrange("p (t b o) -> p t b o", b=NB, o=OB)

            sgA0 = statep.tile([128, 8], f32, tag="sgA0")
            sgA1 = statep.tile([128, 8], f32, tag="sgA1")
            sgB0 = statep.tile([128, 8], f32, tag="sgB0")
            sgB1 = statep.tile([128, 8], f32, tag="sgB1")
            sg = [[sgA0, sgA1], [sgB0, sgB1]]
            s09 = statep.tile([128, 8], f32, tag="s09")   # Pool scratch: 0.9*sigma
            for q in range(2):
                nc.vector.memset(sg[q][0][:], 10.0)       # sigma = 10*thr, thr0=1
                nc.vector.memset(mega[q][:, MBC[0]:MBC[0] + 8], 0.0)   # mem0 = 0

            def emit_phase1(ch, sink):
                """Emit phase-1 work units for chunk ch into sink (list of
                thunks); caller interleaves them between steps."""
                c0 = ch * TCH * NB
                xts = []
                for k in range(KB):
                    xr = xinp.tile([128, TCH * NB], f32, tag="xraw")
                    xk = xinp.tile([128, TCH * NB], f32, tag="x")
                    def dma(xr=xr, k=k):
                        nc.gpsimd.dma_start(xr[:], xt[k * 128:(k + 1) * 128, c0:c0 + TCH * NB])
                    def cp(xr=xr, xk=xk):
                        nc.scalar.copy(xk[:].bitcast(f32r), xr[:])
                    sink.append(dma)
                    sink.append(cp)
                    xts.append(xk)
                # 256-col matmul pieces so each fits a PE idle window between
                # recurrence steps without delaying them; pc-major order +
                # per-piece ACT copies so the chunk's first steps unblock
                # after ~1/NPC of the work
                PC = min(256, TCH * NB)
                NPC = (TCH * NB) // PC
                TPP = PC // NB           # timesteps covered per piece
                pss = []
                for ob in range(OB):
                    ps = ps1p.tile([128, TCH * NB], f32, tag="mmps")
                    pss.append(ps)
                for pc in range(NPC):
                    for ob in range(OB):
                        ps = pss[ob]
                        for k in range(KB):
                            def mm(ps=ps, k=k, ob=ob, xk=xts[k], pc=pc):
                                nc.tensor.matmul(
                                    ps[:, pc * PC:(pc + 1) * PC],
                                    w_sb[:, k * O + ob * 128: k * O + ob * 128 + 128].bitcast(f32r),
                                    xk[:, pc * PC:(pc + 1) * PC].bitcast(f32r),
                                    start=(k == 0),
                                    stop=(k == KB - 1),
                                )
                            sink.append(mm)
                        def cpy(ps=ps, ob=ob, ch=ch, pc=pc):
                            src = ps[:, pc * PC:(pc + 1) * PC].rearrange(
                                "p (t b) -> p t b", b=NB)
                            dst = proj4[:, ch * TCH + pc * TPP:
                                        ch * TCH + (pc + 1) * TPP, :, ob]
                            nc.scalar.activation(
                                dst, src, Act.Identity,
                                bias=br_sb[:, ob:ob + 1], scale=1.0,
                            )
                        sink.append(cpy)

            # chunk 0 phase 1 runs up front
            pre = []
            emit_phase1(0, pre)
            for f in pre:
                f()

            # ---- main loop ----
            for ch in range(NCH):
                par = ch % 2
                outc = OUTC[par]
                mbc = MBC[par]
                # next chunk's phase-1, interleaved between steps
                nxt = []
                if ch + 1 < NCH:
                    emit_phase1(ch + 1, nxt)
                nxt_per_step = -(-len(nxt) // TCH) if nxt else 0

                for tl in range(TCH):
                    t = ch * TCH + tl
                    # source column of mem state
                    if tl == 0:
                        mcol = MBC[(ch - 1) % 2] + TCH * 8 if ch > 0 else MBC[0]
                    else:
                        mcol = mbc + tl * 8
                    for q in range(2):
                        mg = mega[q]
                        s_cur = sg[q][t % 2]
                        ps = ps2p.tile([128, 8], f32, tag="u")
                        qc = q * 8
                        # u = proj[t] (+psum) ...
                        nc.tensor.matmul(
                            ps[:], eye1, proj[:, t * 16 + qc: t * 16 + qc + 8],
                            start=True, stop=False,
                        )
                        # ... - 0.1 * sum_o(mem)  (4 o-block reduce matmuls,
                        #     col (b2,k) read dup x4 via stride tricks)
                        for k in range(OB):
                            mv = mg[:, mcol + k: mcol + k + 5: 4]
                            mv = mv.rearrange("p (b x) -> p b x", x=1).broadcast_to((128, 2, 4))
                            nc.tensor.matmul(ps[:], ones_sb[:], mv,
                                             start=False, stop=False)
                        # ... + 0.9 * mem
                        nc.tensor.matmul(
                            ps[:], eye9, mg[:, mcol: mcol + 8],
                            start=False, stop=True,
                        )
                        # pages: 0 -> spike @ OUT slot tl, 1 -> mem' @ MB slot tl+1
                        outap = mg[:, outc + tl * 8: outc + tl * 8 + 8]
                        outap = outap.rearrange("p (x c) -> p x c", x=1).copy()
                        outap.ap[1] = (DPAGE, 2)
                        in0 = ps[:].rearrange("p (x c) -> p x c", x=1).broadcast_to((128, 2, 8))
                        in1 = s_cur[:].rearrange("p (x c) -> p x c", x=1).broadcast_to((128, 2, 8))
                        nc.vector._custom_dve(
                            PAIRFIRE, out=outap, in0=in0, in1=in1, s1=INHIB,
                        )
                    # sigma' = 0.9*sigma + spk (DVE stt: same-engine in-order
                    # after both pair ops -> zero sem cost, hides in latency)
                    for q in range(2):
                        spk = mega[q][:, outc + tl * 8: outc + tl * 8 + 8]
                        nc.vector.scalar_tensor_tensor(
                            sg[q][(t + 1) % 2][:], sg[q][t % 2][:], DECAY,
                            spk, Alu.mult, Alu.add,
                        )
                    # interleave next chunk's phase-1 work
                    for _ in range(nxt_per_step):
                        if nxt:
                            nxt.pop(0)()
                for f in nxt:
                    f()
                # stream spikes of this chunk out (per-chain contiguous regions)
                for q in range(2):
                    nc.gpsimd.dma_start(
                        out_d[:, q * n_steps * 8 + ch * OUTW:
                              q * n_steps * 8 + (ch + 1) * OUTW],
                        mega[q][:, outc:outc + OUTW],
                    )

    nc.compile()
    return nc


def _get_nc(n_steps):
    if n_steps not in _CACHE:
        _CACHE[n_steps] = _build(n_steps)
    return _CACHE[n_steps]


def _host_pack(x, W, b, n_steps):
    xs_all = []
    for c in range(8):
        xs = x[c * NB:(c + 1) * NB, :n_steps, :]          # [NB, S, I]
        xtc = np.ascontiguousarray(xs.transpose(2, 1, 0)).reshape(I, n_steps * NB)
        xs_all.append(xtc)
    wt = np.ascontiguousarray(W.T)                         # [I, O]
    br = np.ascontiguousarray(b.reshape(4, 128).T)         # [128, 4]
    eye = np.eye(128, dtype=np.float32)
    eyes = np.concatenate([eye, (np.float32(DECAY) * eye)], axis=1)
    return [{"xt": xs_all[c], "wt": wt, "br": br, "eyes": eyes} for c in range(8)]


def _host_unpack(outs, n_steps):
    full = np.empty((B, n_steps, O), dtype=np.float32)
    for c in range(8):
        o = outs[c]["out"].reshape(128, 2, n_steps, 2, 4)  # [o_in, q, t, b2, k]
        full[c * NB:(c + 1) * NB] = np.ascontiguousarray(
            o.transpose(1, 3, 2, 4, 0)).reshape(NB, n_steps, O)
    return full


def kernel(x, W, b, n_steps=S, trace=False):
    from concourse.bass_utils import run_bass_kernel_spmd

    x = np.asarray(x, dtype=np.float32)
    W = np.asarray(W, dtype=np.float32)
    b = np.asarray(b, dtype=np.float32)
    nc = _get_nc(n_steps)
    in_maps = _host_pack(x, W, b, n_steps)
    res = run_bass_kernel_spmd(nc, in_maps, core_ids=list(range(8)), trace=trace)
    out = _host_unpack(res.results, n_steps)
    kernel.last_result = res
    return out


# revision 11
# speedup vs baseline: 1.0016x; 1.0016x over previous
"""AdaptiveResonateAndFireNeuron Trainium2 kernel v2 (8 cores, data-parallel batch).

Per batch row b:
  proj[t] = x[b,t,:] @ W.T + bias
  u       = 0.9*mem + proj[t] - 0.1*sum_o(mem)
  spk     = (u >= thr);  mem' = u - spk*thr;  thr' = 0.9*thr + 0.1*spk

v2 structure (vs v1 baseline):
- phase 1 (proj) matmuls in float32r (1 cyc/row at 512 cols vs 4 for fp32);
  PSUM->SBUF copy + bias moved from DVE to the Activation engine.
- phase 2: u is accumulated ENTIRELY in PSUM by 6 matmuls per step per chain
  (identity@proj + 4x (-0.1*ones)@mem [o-block reduce] + 0.9*identity@mem).
  A single custom DVE op (ARF_PAIRFIRE, subdim pages) then reads PSUM once and
  writes BOTH outputs: page0 = spike (straight into the DMA staging buffer),
  page1 = mem' (into the mem trajectory ring).  Threshold state is kept scaled
  (sigma = 10*thr) and updated on the GpSimd/Pool engine with ONE
  scalar_tensor_tensor: sigma' = 0.9*sigma + spk.
- two independent chains (2 batches each) interleave so the PE->DVE->PE
  latency of one chain hides under the other.

Core layout: partition p = o_in (o % 128); per-step col c = b4*4 + k where
b4 = local batch (0..3), k = o_blk (o // 128); chain q = b4 // 2.
"""

import numpy as np

B, S, I, O = 32, 1024, 512, 512

SIGMA_ENGINE = "gpsimd"          # "gpsimd" (Pool) or "vector" (DVE)


def _SIGMA_ENG(nc):
    return getattr(nc, SIGMA_ENGINE)
NB = B // 8          # batches per core
DECAY = 0.9
INHIB = 0.1

_CACHE = {}


def _register_dve_ops():
    import concourse.dve_ops as dve_ops
    from concourse.dve_spec import Spec, Src0, Src1, C1, select, minn, SubIdx
    from concourse.dve_table_gen import dve_ver_for

    if "ARF_PAIRFIRE" in dve_ops._SUB_OPCODE_FOR_NAME:
        from concourse.dve_ops import OPS
        by_name = {op.name: op for op in OPS}
        return by_name["ARF_PAIRFIRE"]

    # in0 = u (PSUM, page-broadcast), in1 = sigma = 10*thr (page-broadcast),
    # s1 (C1) = 0.1.  h = 0.1*sigma = thr.
    # page 0 (SubIdx falsy) -> spike = (u >= thr)
    # page 1                -> mem'  = min(u, u - thr)   (valid: thr > 0)
    h = Src1 * C1
    body = select(SubIdx, minn(Src0, Src0 - h), Src0 >= h)

    def ref(in0, in1, s0, s1, imm2):
        u = in0.astype(np.float32)
        thr = (in1 * np.float32(s1)).astype(np.float32)
        spk = (u >= thr).astype(np.float32)
        memf = np.minimum(u, (u - thr).astype(np.float32))
        pages = np.arange(in0.shape[1])[None, :, None]
        return np.where(pages == 0, spk, memf).astype(np.float32)

    spec = Spec(body=body, reference=ref)
    op = dve_ops.DveOp("ARF_PAIRFIRE", spec, subdim=True, uops_sha={})
    dve_ops.OPS.append(op)
    dve_ops.CUSTOM_DVE_SPECS["ARF_PAIRFIRE"] = spec
    dve_ops._SUB_OPCODE_FOR_NAME["ARF_PAIRFIRE"] = (
        dve_ops._CUSTOM_DVE_ROW_BASE + len(dve_ops.OPS) - 1
    )
    ver = dve_ver_for("TRN2")
    try:
        op.compile(ver)
    except ValueError as e:
        import re

        m = re.search(r":\s*([0-9a-f]{8,})\s*≠", str(e))
        if m is None:
            raise
        op.uops_sha[ver] = m.group(1)
        dve_ops._COMPILE_CACHE.pop(("ARF_PAIRFIRE", ver), None)
        op.compile(ver)
    return op


def _build(n_steps):
    import concourse.bass as bass
    from concourse import bacc
    import concourse.mybir as mybir
    from concourse import tile

    PAIRFIRE = _register_dve_ops()

    f32 = mybir.dt.float32
    f32r = mybir.dt.float32r
    Alu = mybir.AluOpType
    Act = mybir.ActivationFunctionType

    nc = bacc.Bacc()
    # xt[i, t*4+b4] = x[b4, t, i]  (host pre-transposed, per core)
    xt = nc.declare_dram_parameter("xt", [I, NB * n_steps], f32, isOutput=False)
    # cst: [w blocks (4x512) | br (4) | eyes (256)] -- single constants DMA
    # cst[:, k*512+o] = W[o, k*128+p]; cst[:, 2048+k] = bias[k*128+p]
    # cst[:, 2052:2180] = I; cst[:, 2180:2308] = 0.9*I
    cst = nc.declare_dram_parameter("cst", [128, 2308], f32, isOutput=False)
    # out[o_in, q*(n_steps*8) + t*8 + b2*4 + k] = spikes[q*2+b2, t, k*128+o_in]
    out_d = nc.declare_dram_parameter("out", [128, n_steps * 16], f32, isOutput=True)

    KB = I // 128            # 4 contraction blocks
    OB = O // 128            # 4 output blocks
    TCH = min(128, n_steps)  # timesteps per chunk
    assert n_steps % TCH == 0
    NCH = n_steps // TCH

    # per-chain mega-tile column layout: [OUT0 | MB0 | OUT1 | MB1]
    # (separate tile per chain so hazard intervals never cross chains)
    OUTW = TCH * 8
    MBW = (TCH + 1) * 8
    OUTC = [0, OUTW + MBW]
    MBC = [OUTW, 2 * OUTW + MBW]
    MEGA_W = 2 * (OUTW + MBW)
    DPAGE = MBC[0] - OUTC[0] + 8      # +8: mem slot t+1 vs spike slot t

    with tile.TileContext(nc) as tc:
        with (
            tc.tile_pool(name="const", bufs=1) as constp,
            tc.tile_pool(name="state", bufs=1) as statep,
            tc.tile_pool(name="big", bufs=1) as bigp,
            tc.tile_pool(name="xin", bufs=4) as xinp,
            tc.tile_pool(name="ps1", bufs=4, space=bass.MemorySpace.PSUM) as ps1p,
            tc.tile_pool(name="ps2", bufs=4, space=bass.MemorySpace.PSUM) as ps2p,
        ):
            # ---- constants (one DMA + ACT proxy copies so PE waits on one sem) ----
            c_raw = constp.tile([128, 2308], f32, tag="craw")
            w_sb = constp.tile([128, KB * O], f32, tag="w")
            br_sb = constp.tile([128, 4], f32, tag="br")
            ey_sb = constp.tile([128, 256], f32, tag="ey")
            ones_sb = constp.tile([128, 128], f32, tag="ones")
            warm = constp.tile([128, 1], f32, tag="warm")
            # tiny ACT op up front: act-table load overlaps the DMAs
            nc.vector.memset(warm[:], 0.0)
            nc.scalar.activation(warm[:], warm[:], Act.Identity, bias=0.0, scale=1.0)
            nc.gpsimd.dma_start(c_raw[:], cst[:])
            nc.scalar.copy(w_sb[:].bitcast(f32r), c_raw[:, 0:KB * O])
            nc.scalar.copy(br_sb[:], c_raw[:, KB * O:KB * O + 4])
            nc.scalar.copy(ey_sb[:], c_raw[:, KB * O + 4:KB * O + 260])
            nc.vector.memset(ones_sb[:], -INHIB)
            eye1 = ey_sb[:, 0:128]
            eye9 = ey_sb[:, 128:256]

            # ---- persistent buffers ----
            proj = bigp.tile([128, n_steps * 16], f32, tag="proj")
            megaA = bigp.tile([128, MEGA_W], f32, tag="megaA")
            megaB = bigp.tile([128, MEGA_W], f32, tag="megaB")
            mega = [megaA, megaB]
            proj4 = proj[:].rearrange("p (t b o) -> p t b o", b=NB, o=OB)

            sgA0 = statep.tile([128, 8], f32, tag="sgA0")
            sgA1 = statep.tile([128, 8], f32, tag="sgA1")
            sgB0 = statep.tile([128, 8], f32, tag="sgB0")
            sgB1 = statep.tile([128, 8], f32, tag="sgB1")
            sg = [[sgA0, sgA1], [sgB0, sgB1]]
            s09 = statep.tile([128, 8], f32, tag="s09")   # Pool scratch: 0.9*sigma
            for q in range(2):
                nc.vector.memset(sg[q][0][:], 10.0)       # sigma = 10*thr, thr0=1
                nc.vector.memset(mega[q][:, MBC[0]:MBC[0] + 8], 0.0)   # mem0 = 0

            def emit_phase1(ch, sink):
                """Emit phase-1 work units for chunk ch into sink (list of
                thunks); caller interleaves them between steps."""
                c0 = ch * TCH * NB
                xr = xinp.tile([128, KB * TCH * NB], f32, tag="xraw")
                xa = xinp.tile([128, KB * TCH * NB], f32, tag="x")
                xoff = [k * TCH * NB for k in range(KB)]
                for k in range(KB):
                    def dma(xr=xr, c0=c0, k=k):
                        nc.gpsimd.dma_start(
                            xr[:, xoff[k]:xoff[k] + TCH * NB],
                            xt[k * 128:(k + 1) * 128, c0:c0 + TCH * NB])
                    sink.append(dma)
                # 256-col matmul pieces so each fits a PE idle window between
                # recurrence steps without delaying them; pc-major order +
                # per-piece ACT copies so the chunk's first steps unblock
                # after ~1/NPC of the work
                PC = min(256, TCH * NB)
                NPC = (TCH * NB) // PC
                TPP = PC // NB           # timesteps covered per piece
                pss = []
                for ob in range(OB):
                    ps = ps1p.tile([128, TCH * NB], f32, tag="mmps")
                    pss.append(ps)
                for pc in range(NPC):
                    # fp32r-rounding proxy copies for this piece's x columns
                    for k in range(KB):
                        def xcp(xr=xr, xa=xa, xo=xoff[k], pc=pc):
                            nc.scalar.copy(
                                xa[:, xo + pc * PC: xo + (pc + 1) * PC].bitcast(f32r),
                                xr[:, xo + pc * PC: xo + (pc + 1) * PC])
                        sink.append(xcp)
                    for ob in range(OB):
                        ps = pss[ob]
                        for k in range(KB):
                            def mm(ps=ps, k=k, ob=ob, xo=xoff[k], xa=xa, pc=pc):
                                nc.tensor.matmul(
                                    ps[:, pc * PC:(pc + 1) * PC],
                                    w_sb[:, k * O + ob * 128: k * O + ob * 128 + 128].bitcast(f32r),
                                    xa[:, xo + pc * PC: xo + (pc + 1) * PC].bitcast(f32r),
                                    start=(k == 0),
                                    stop=(k == KB - 1),
                                )
                            sink.append(mm)
                        def cpy(ps=ps, ob=ob, ch=ch, pc=pc):
                            src = ps[:, pc * PC:(pc + 1) * PC].rearrange(
                                "p (t b) -> p t b", b=NB)
                            dst = proj4[:, ch * TCH + pc * TPP:
                                        ch * TCH + (pc + 1) * TPP, :, ob]
                            nc.scalar.activation(
                                dst, src, Act.Identity,
                                bias=br_sb[:, ob:ob + 1], scale=1.0,
                            )
                        sink.append(cpy)

            # chunk 0 phase 1 runs up front
            pre = []
            emit_phase1(0, pre)
            for f in pre:
                f()

            # ---- main loop ----
            for ch in range(NCH):
                par = ch % 2
                outc = OUTC[par]
                mbc = MBC[par]
                # next chunk's phase-1, interleaved between steps
                nxt = []
                if ch + 1 < NCH:
                    emit_phase1(ch + 1, nxt)
                nxt_per_step = -(-len(nxt) // TCH) if nxt else 0

                for tl in range(TCH):
                    t = ch * TCH + tl
                    # source column of mem state
                    if tl == 0:
                        mcol = MBC[(ch - 1) % 2] + TCH * 8 if ch > 0 else MBC[0]
                    else:
                        mcol = mbc + tl * 8
                    for q in range(2):
                        mg = mega[q]
                        s_cur = sg[q][t % 2]
                        ps = ps2p.tile([128, 8], f32, tag="u")
                        qc = q * 8
                        # u = proj[t] (+psum) ...
                        nc.tensor.matmul(
                            ps[:], eye1, proj[:, t * 16 + qc: t * 16 + qc + 8],
                            start=True, stop=False,
                        )
                        # ... - 0.1 * sum_o(mem)  (4 o-block reduce matmuls,
                        #     col (b2,k) read dup x4 via stride tricks)
                        for k in range(OB):
                            mv = mg[:, mcol + k: mcol + k + 5: 4]
                            mv = mv.rearrange("p (b x) -> p b x", x=1).broadcast_to((128, 2, 4))
                            nc.tensor.matmul(ps[:], ones_sb[:], mv,
                                             start=False, stop=False)
                        # ... + 0.9 * mem
                        nc.tensor.matmul(
                            ps[:], eye9, mg[:, mcol: mcol + 8],
                            start=False, stop=True,
                        )
                        # pages: 0 -> spike @ OUT slot tl, 1 -> mem' @ MB slot tl+1
                        outap = mg[:, outc + tl * 8: outc + tl * 8 + 8]
                        outap = outap.rearrange("p (x c) -> p x c", x=1).copy()
                        outap.ap[1] = (DPAGE, 2)
                        in0 = ps[:].rearrange("p (x c) -> p x c", x=1).broadcast_to((128, 2, 8))
                        in1 = s_cur[:].rearrange("p (x c) -> p x c", x=1).broadcast_to((128, 2, 8))
                        nc.vector._custom_dve(
                            PAIRFIRE, out=outap, in0=in0, in1=in1, s1=INHIB,
                        )
                    # sigma' = 0.9*sigma + spk (DVE stt: same-engine in-order
                    # after both pair ops -> zero sem cost, hides in latency)
                    for q in range(2):
                        spk = mega[q][:, outc + tl * 8: outc + tl * 8 + 8]
                        nc.vector.scalar_tensor_tensor(
                            sg[q][(t + 1) % 2][:], sg[q][t % 2][:], DECAY,
                            spk, Alu.mult, Alu.add,
                        )
                    # interleave next chunk's phase-1 work
                    for _ in range(nxt_per_step):
                        if nxt:
                            nxt.pop(0)()
                for f in nxt:
                    f()
                # stream spikes of this chunk out (per-chain contiguous regions)
                for q in range(2):
                    nc.gpsimd.dma_start(
                        out_d[:, q * n_steps * 8 + ch * OUTW:
                              q * n_steps * 8 + (ch + 1) * OUTW],
                        mega[q][:, outc:outc + OUTW],
                    )

    nc.compile()
    return nc


def _get_nc(n_steps):
    if n_steps not in _CACHE:
        _CACHE[n_steps] = _build(n_steps)
    return _CACHE[n_steps]


def _host_pack(x, W, b, n_steps):
    xs_all = []
    for c in range(8):
        xs = x[c * NB:(c + 1) * NB, :n_steps, :]          # [NB, S, I]
        xtc = np.ascontiguousarray(xs.transpose(2, 1, 0)).reshape(I, n_steps * NB)
        xs_all.append(xtc)
    wt = np.ascontiguousarray(W.T)                         # [I, O]
    wblk = np.concatenate([wt[k * 128:(k + 1) * 128] for k in range(4)], axis=1)
    br = np.ascontiguousarray(b.reshape(4, 128).T)         # [128, 4]
    eye = np.eye(128, dtype=np.float32)
    cst = np.concatenate(
        [wblk, br, eye, (np.float32(DECAY) * eye)], axis=1
    ).astype(np.float32)                                   # [128, 2308]
    return [{"xt": xs_all[c], "cst": cst} for c in range(8)]


def _host_unpack(outs, n_steps):
    full = np.empty((B, n_steps, O), dtype=np.float32)
    for c in range(8):
        o = outs[c]["out"].reshape(128, 2, n_steps, 2, 4)  # [o_in, q, t, b2, k]
        full[c * NB:(c + 1) * NB] = np.ascontiguousarray(
            o.transpose(1, 3, 2, 4, 0)).reshape(NB, n_steps, O)
    return full


def kernel(x, W, b, n_steps=S, trace=False):
    from concourse.bass_utils import run_bass_kernel_spmd

    x = np.asarray(x, dtype=np.float32)
    W = np.asarray(W, dtype=np.float32)
    b = np.asarray(b, dtype=np.float32)
    nc = _get_nc(n_steps)
    in_maps = _host_pack(x, W, b, n_steps)
    res = run_bass_kernel_spmd(nc, in_maps, core_ids=list(range(8)), trace=trace)
    out = _host_unpack(res.results, n_steps)
    kernel.last_result = res
    return out


# revision 23
# speedup vs baseline: 1.0051x; 1.0035x over previous
"""AdaptiveResonateAndFireNeuron Trainium2 kernel v2 (8 cores, data-parallel batch).

Per batch row b:
  proj[t] = x[b,t,:] @ W.T + bias
  u       = 0.9*mem + proj[t] - 0.1*sum_o(mem)
  spk     = (u >= thr);  mem' = u - spk*thr;  thr' = 0.9*thr + 0.1*spk

v2 structure (vs v1 baseline):
- phase 1 (proj) matmuls in float32r (1 cyc/row at 512 cols vs 4 for fp32);
  PSUM->SBUF copy + bias moved from DVE to the Activation engine.
- phase 2: u is accumulated ENTIRELY in PSUM by 6 matmuls per step per chain
  (identity@proj + 4x (-0.1*ones)@mem [o-block reduce] + 0.9*identity@mem).
  A single custom DVE op (ARF_PAIRFIRE, subdim pages) then reads PSUM once and
  writes BOTH outputs: page0 = spike (straight into the DMA staging buffer),
  page1 = mem' (into the mem trajectory ring).  Threshold state is kept scaled
  (sigma = 10*thr) and updated on the GpSimd/Pool engine with ONE
  scalar_tensor_tensor: sigma' = 0.9*sigma + spk.
- two independent chains (2 batches each) interleave so the PE->DVE->PE
  latency of one chain hides under the other.

Core layout: partition p = o_in (o % 128); per-step col c = b4*4 + k where
b4 = local batch (0..3), k = o_blk (o // 128); chain q = b4 // 2.
"""

import numpy as np

B, S, I, O = 32, 1024, 512, 512

SIGMA_ENGINE = "gpsimd"          # "gpsimd" (Pool) or "vector" (DVE)


def _SIGMA_ENG(nc):
    return getattr(nc, SIGMA_ENGINE)
NB = B // 8          # batches per core
DECAY = 0.9
INHIB = 0.1

_CACHE = {}


def _register_dve_ops():
    import concourse.dve_ops as dve_ops
    from concourse.dve_spec import Spec, Src0, Src1, C1, select, minn, SubIdx
    from concourse.dve_table_gen import dve_ver_for

    if "ARF_PAIRFIRE" in dve_ops._SUB_OPCODE_FOR_NAME:
        from concourse.dve_ops import OPS
        by_name = {op.name: op for op in OPS}
        return by_name["ARF_PAIRFIRE"]

    # in0 = u (PSUM, page-broadcast), in1 = sigma = 10*thr (page-broadcast),
    # s1 (C1) = 0.1.  h = 0.1*sigma = thr.
    # page 0 (SubIdx falsy) -> spike = (u >= thr)
    # page 1                -> mem'  = min(u, u - thr)   (valid: thr > 0)
    h = Src1 * C1
    body = select(SubIdx, minn(Src0, Src0 - h), Src0 >= h)

    def ref(in0, in1, s0, s1, imm2):
        u = in0.astype(np.float32)
        thr = (in1 * np.float32(s1)).astype(np.float32)
        spk = (u >= thr).astype(np.float32)
        memf = np.minimum(u, (u - thr).astype(np.float32))
        pages = np.arange(in0.shape[1])[None, :, None]
        return np.where(pages == 0, spk, memf).astype(np.float32)

    spec = Spec(body=body, reference=ref)
    op = dve_ops.DveOp("ARF_PAIRFIRE", spec, subdim=True, uops_sha={})
    dve_ops.OPS.append(op)
    dve_ops.CUSTOM_DVE_SPECS["ARF_PAIRFIRE"] = spec
    dve_ops._SUB_OPCODE_FOR_NAME["ARF_PAIRFIRE"] = (
        dve_ops._CUSTOM_DVE_ROW_BASE + len(dve_ops.OPS) - 1
    )
    ver = dve_ver_for("TRN2")
    try:
        op.compile(ver)
    except ValueError as e:
        import re

        m = re.search(r":\s*([0-9a-f]{8,})\s*≠", str(e))
        if m is None:
            raise
        op.uops_sha[ver] = m.group(1)
        dve_ops._COMPILE_CACHE.pop(("ARF_PAIRFIRE", ver), None)
        op.compile(ver)
    return op


def _build(n_steps):
    import concourse.bass as bass
    from concourse import bacc
    import concourse.mybir as mybir
    from concourse import tile

    PAIRFIRE = _register_dve_ops()

    f32 = mybir.dt.float32
    f32r = mybir.dt.float32r
    Alu = mybir.AluOpType
    Act = mybir.ActivationFunctionType

    nc = bacc.Bacc()
    # xt[i, t*4+b4] = x[b4, t, i]  (host pre-transposed, per core)
    xt = nc.declare_dram_parameter("xt", [I, NB * n_steps], f32, isOutput=False)
    # cst: [w blocks (4x512) | br (4) | eyes (256)] -- single constants DMA
    # cst[:, k*512+o] = W[o, k*128+p]; cst[:, 2048+k] = bias[k*128+p]
    # cst[:, 2052:2180] = I; cst[:, 2180:2308] = 0.9*I
    cst = nc.declare_dram_parameter("cst", [128, 2308], f32, isOutput=False)
    # out[o_in, q*(n_steps*8) + t*8 + b2*4 + k] = spikes[q*2+b2, t, k*128+o_in]
    out_d = nc.declare_dram_parameter("out", [128, n_steps * 16], f32, isOutput=True)

    KB = I // 128            # 4 contraction blocks
    OB = O // 128            # 4 output blocks
    TCH = min(128, n_steps)  # timesteps per chunk
    assert n_steps % TCH == 0
    NCH = n_steps // TCH

    # per-chain mega-tile column layout: [OUT0 | MB0 | OUT1 | MB1]
    # (separate tile per chain so hazard intervals never cross chains)
    OUTW = TCH * 8
    MBW = (TCH + 1) * 8
    OUTC = [0, OUTW + MBW]
    MBC = [OUTW, 2 * OUTW + MBW]
    MEGA_W = 2 * (OUTW + MBW)
    DPAGE = MBC[0] - OUTC[0] + 8      # +8: mem slot t+1 vs spike slot t

    with tile.TileContext(nc) as tc:
        with (
            tc.tile_pool(name="const", bufs=1) as constp,
            tc.tile_pool(name="state", bufs=1) as statep,
            tc.tile_pool(name="big", bufs=1) as bigp,
            tc.tile_pool(name="xin", bufs=4) as xinp,
            tc.tile_pool(name="ps1", bufs=4, space=bass.MemorySpace.PSUM) as ps1p,
            tc.tile_pool(name="ps2", bufs=4, space=bass.MemorySpace.PSUM) as ps2p,
        ):
            # ---- constants (one DMA + ACT proxy copies so PE waits on one sem) ----
            c_raw = constp.tile([128, 2308], f32, tag="craw")
            w_sb = constp.tile([128, KB * O], f32, tag="w")
            br_sb = constp.tile([128, 4], f32, tag="br")
            ey_sb = constp.tile([128, 256], f32, tag="ey")
            ones_sb = constp.tile([128, 128], f32, tag="ones")
            warm = constp.tile([128, 1], f32, tag="warm")
            # tiny ACT op up front: act-table load overlaps the DMAs
            nc.vector.memset(warm[:], 0.0)
            nc.scalar.activation(warm[:], warm[:], Act.Identity, bias=0.0, scale=1.0)
            # per-k W DMA + copy thunks; interleaved with the x DMAs below so
            # the k=0 matmuls start after just two transfers
            def w_load(k):
                nc.gpsimd.dma_start(c_raw[:, k * O:(k + 1) * O],
                                    cst[:, k * O:(k + 1) * O])
                nc.scalar.copy(w_sb[:, k * O:(k + 1) * O].bitcast(f32r),
                               c_raw[:, k * O:(k + 1) * O])

            def emit_const_tail():
                nc.gpsimd.dma_start(c_raw[:, KB * O:], cst[:, KB * O:])
                nc.scalar.copy(br_sb[:], c_raw[:, KB * O:KB * O + 4])
                nc.scalar.copy(ey_sb[:], c_raw[:, KB * O + 4:KB * O + 260])

            nc.vector.memset(ones_sb[:], -INHIB)
            eye1 = ey_sb[:, 0:128]
            eye9 = ey_sb[:, 128:256]

            # ---- persistent buffers ----
            proj = bigp.tile([128, n_steps * 16], f32, tag="proj")
            megaA = bigp.tile([128, MEGA_W], f32, tag="megaA")
            megaB = bigp.tile([128, MEGA_W], f32, tag="megaB")
            mega = [megaA, megaB]
            proj4 = proj[:].rearrange("p (t b o) -> p t b o", b=NB, o=OB)

            sgA0 = statep.tile([128, 8], f32, tag="sgA0")
            sgA1 = statep.tile([128, 8], f32, tag="sgA1")
            sgB0 = statep.tile([128, 8], f32, tag="sgB0")
            sgB1 = statep.tile([128, 8], f32, tag="sgB1")
            sg = [[sgA0, sgA1], [sgB0, sgB1]]
            s09 = statep.tile([128, 8], f32, tag="s09")   # Pool scratch: 0.9*sigma
            for q in range(2):
                nc.vector.memset(sg[q][0][:], 10.0)       # sigma = 10*thr, thr0=1
                nc.vector.memset(mega[q][:, MBC[0]:MBC[0] + 8], 0.0)   # mem0 = 0

            def emit_phase1(ch, sink):
                """Emit phase-1 work units for chunk ch into sink (list of
                thunks); caller interleaves them between steps."""
                c0 = ch * TCH * NB
                xr = xinp.tile([128, KB * TCH * NB], f32, tag="xraw")
                xa = xinp.tile([128, KB * TCH * NB], f32, tag="x")
                xoff = [k * TCH * NB for k in range(KB)]
                for k in range(KB):
                    def dma(xr=xr, c0=c0, k=k):
                        nc.gpsimd.dma_start(
                            xr[:, xoff[k]:xoff[k] + TCH * NB],
                            xt[k * 128:(k + 1) * 128, c0:c0 + TCH * NB])
                    sink.append(dma)
                # 256-col matmul pieces so each fits a PE idle window between
                # recurrence steps without delaying them; pc-major order +
                # per-piece ACT copies so the chunk's first steps unblock
                # after ~1/NPC of the work
                PC = min(256, TCH * NB)
                NPC = (TCH * NB) // PC
                TPP = PC // NB           # timesteps covered per piece
                pss = []
                for ob in range(OB):
                    ps = ps1p.tile([128, TCH * NB], f32, tag="mmps")
                    pss.append(ps)
                for pc in range(NPC):
                    # fp32r-rounding proxy copies for this piece's x columns
                    for k in range(KB):
                        def xcp(xr=xr, xa=xa, xo=xoff[k], pc=pc):
                            nc.scalar.copy(
                                xa[:, xo + pc * PC: xo + (pc + 1) * PC].bitcast(f32r),
                                xr[:, xo + pc * PC: xo + (pc + 1) * PC])
                        sink.append(xcp)
                    for ob in range(OB):
                        ps = pss[ob]
                        for k in range(KB):
                            def mm(ps=ps, k=k, ob=ob, xo=xoff[k], xa=xa, pc=pc):
                                nc.tensor.matmul(
                                    ps[:, pc * PC:(pc + 1) * PC],
                                    w_sb[:, k * O + ob * 128: k * O + ob * 128 + 128].bitcast(f32r),
                                    xa[:, xo + pc * PC: xo + (pc + 1) * PC].bitcast(f32r),
                                    start=(k == 0),
                                    stop=(k == KB - 1),
                                )
                            sink.append(mm)
                        def cpy(ps=ps, ob=ob, ch=ch, pc=pc):
                            src = ps[:, pc * PC:(pc + 1) * PC].rearrange(
                                "p (t b) -> p t b", b=NB)
                            dst = proj4[:, ch * TCH + pc * TPP:
                                        ch * TCH + (pc + 1) * TPP, :, ob]
                            nc.scalar.activation(
                                dst, src, Act.Identity,
                                bias=br_sb[:, ob:ob + 1], scale=1.0,
                            )
                        sink.append(cpy)

            # chunk 0 phase 1 runs up front; W and x transfers interleaved,
            # the small br/eyes constants last (not needed until step 0)
            pre = []
            emit_phase1(0, pre)
            for k in range(KB):
                w_load(k)
                pre[k]()                # x_k DMA
            emit_const_tail()
            for f in pre[KB:]:
                f()

            # ---- main loop ----
            for ch in range(NCH):
                par = ch % 2
                outc = OUTC[par]
                mbc = MBC[par]
                # next chunk's phase-1, interleaved between steps
                nxt = []
                if ch + 1 < NCH:
                    emit_phase1(ch + 1, nxt)
                nxt_per_step = -(-len(nxt) // TCH) if nxt else 0

                for tl in range(TCH):
                    t = ch * TCH + tl
                    # source column of mem state
                    if tl == 0:
                        mcol = MBC[(ch - 1) % 2] + TCH * 8 if ch > 0 else MBC[0]
                    else:
                        mcol = mbc + tl * 8
                    for q in range(2):
                        mg = mega[q]
                        s_cur = sg[q][t % 2]
                        ps = ps2p.tile([128, 8], f32, tag="u")
                        qc = q * 8
                        # u = proj[t] (+psum) ...
                        nc.tensor.matmul(
                            ps[:], eye1, proj[:, t * 16 + qc: t * 16 + qc + 8],
                            start=True, stop=False,
                        )
                        # ... - 0.1 * sum_o(mem)  (4 o-block reduce matmuls,
                        #     col (b2,k) read dup x4 via stride tricks)
                        for k in range(OB):
                            mv = mg[:, mcol + k: mcol + k + 5: 4]
                            mv = mv.rearrange("p (b x) -> p b x", x=1).broadcast_to((128, 2, 4))
                            nc.tensor.matmul(ps[:], ones_sb[:], mv,
                                             start=False, stop=False)
                        # ... + 0.9 * mem
                        nc.tensor.matmul(
                            ps[:], eye9, mg[:, mcol: mcol + 8],
                            start=False, stop=True,
                        )
                        # pages: 0 -> spike @ OUT slot tl, 1 -> mem' @ MB slot tl+1
                        outap = mg[:, outc + tl * 8: outc + tl * 8 + 8]
                        outap = outap.rearrange("p (x c) -> p x c", x=1).copy()
                        outap.ap[1] = (DPAGE, 2)
                        in0 = ps[:].rearrange("p (x c) -> p x c", x=1).broadcast_to((128, 2, 8))
                        in1 = s_cur[:].rearrange("p (x c) -> p x c", x=1).broadcast_to((128, 2, 8))
                        nc.vector._custom_dve(
                            PAIRFIRE, out=outap, in0=in0, in1=in1, s1=INHIB,
                        )
                    # sigma' = 0.9*sigma + spk (DVE stt: same-engine in-order
                    # after both pair ops -> zero sem cost, hides in latency)
                    for q in range(2):
                        spk = mega[q][:, outc + tl * 8: outc + tl * 8 + 8]
                        nc.vector.scalar_tensor_tensor(
                            sg[q][(t + 1) % 2][:], sg[q][t % 2][:], DECAY,
                            spk, Alu.mult, Alu.add,
                        )
                    # interleave next chunk's phase-1 work
                    for _ in range(nxt_per_step):
                        if nxt:
                            nxt.pop(0)()
                    # first half of this chunk's spikes can stream out early
                    # (shrinks the end-of-kernel DMA tail)
                    if tl == TCH // 2 and TCH > 1:
                        for q in range(2):
                            nc.gpsimd.dma_start(
                                out_d[:, q * n_steps * 8 + ch * OUTW:
                                      q * n_steps * 8 + ch * OUTW + OUTW // 2],
                                mega[q][:, outc:outc + OUTW // 2],
                            )
                for f in nxt:
                    f()
                # stream the rest of this chunk's spikes out
                h = OUTW // 2 if TCH > 1 else 0
                for q in range(2):
                    nc.gpsimd.dma_start(
                        out_d[:, q * n_steps * 8 + ch * OUTW + h:
                              q * n_steps * 8 + (ch + 1) * OUTW],
                        mega[q][:, outc + h:outc + OUTW],
                    )

    nc.compile()
    return nc


def _get_nc(n_steps):
    if n_steps not in _CACHE:
        _CACHE[n_steps] = _build(n_steps)
    return _CACHE[n_steps]


def _host_pack(x, W, b, n_steps):
    xs_all = []
    for c in range(8):
        xs = x[c * NB:(c + 1) * NB, :n_steps, :]          # [NB, S, I]
        xtc = np.ascontiguousarray(xs.transpose(2, 1, 0)).reshape(I, n_steps * NB)
        xs_all.append(xtc)
    wt = np.ascontiguousarray(W.T)                         # [I, O]
    wblk = np.concatenate([wt[k * 128:(k + 1) * 128] for k in range(4)], axis=1)
    br = np.ascontiguousarray(b.reshape(4, 128).T)         # [128, 4]
    eye = np.eye(128, dtype=np.float32)
    cst = np.concatenate(
        [wblk, br, eye, (np.float32(DECAY) * eye)], axis=1
    ).astype(np.float32)                                   # [128, 2308]
    return [{"xt": xs_all[c], "cst": cst} for c in range(8)]


def _host_unpack(outs, n_steps):
    full = np.empty((B, n_steps, O), dtype=np.float32)
    for c in range(8):
        o = outs[c]["out"].reshape(128, 2, n_steps, 2, 4)  # [o_in, q, t, b2, k]
        full[c * NB:(c + 1) * NB] = np.ascontiguousarray(
            o.transpose(1, 3, 2, 4, 0)).reshape(NB, n_steps, O)
    return full


def kernel(x, W, b, n_steps=S, trace=False):
    from concourse.bass_utils import run_bass_kernel_spmd

    x = np.asarray(x, dtype=np.float32)
    W = np.asarray(W, dtype=np.float32)
    b = np.asarray(b, dtype=np.float32)
    nc = _get_nc(n_steps)
    in_maps = _host_pack(x, W, b, n_steps)
    res = run_bass_kernel_spmd(nc, in_maps, core_ids=list(range(8)), trace=trace)
    out = _host_unpack(res.results, n_steps)
    kernel.last_result = res
    return out


# revision 30
# speedup vs baseline: 1.0058x; 1.0007x over previous
"""AdaptiveResonateAndFireNeuron Trainium2 kernel v2 (8 cores, data-parallel batch).

Per batch row b:
  proj[t] = x[b,t,:] @ W.T + bias
  u       = 0.9*mem + proj[t] - 0.1*sum_o(mem)
  spk     = (u >= thr);  mem' = u - spk*thr;  thr' = 0.9*thr + 0.1*spk

v2 structure (vs v1 baseline):
- phase 1 (proj) matmuls in float32r (1 cyc/row at 512 cols vs 4 for fp32);
  PSUM->SBUF copy + bias moved from DVE to the Activation engine.
- phase 2: u is accumulated ENTIRELY in PSUM by 6 matmuls per step per chain
  (identity@proj + 4x (-0.1*ones)@mem [o-block reduce] + 0.9*identity@mem).
  A single custom DVE op (ARF_PAIRFIRE, subdim pages) then reads PSUM once and
  writes BOTH outputs: page0 = spike (straight into the DMA staging buffer),
  page1 = mem' (into the mem trajectory ring).  Threshold state is kept scaled
  (sigma = 10*thr) and updated on the GpSimd/Pool engine with ONE
  scalar_tensor_tensor: sigma' = 0.9*sigma + spk.
- two independent chains (2 batches each) interleave so the PE->DVE->PE
  latency of one chain hides under the other.

Core layout: partition p = o_in (o % 128); per-step col c = b4*4 + k where
b4 = local batch (0..3), k = o_blk (o // 128); chain q = b4 // 2.
"""

import numpy as np

B, S, I, O = 32, 1024, 512, 512

SIGMA_ENGINE = "gpsimd"          # "gpsimd" (Pool) or "vector" (DVE)


def _SIGMA_ENG(nc):
    return getattr(nc, SIGMA_ENGINE)
NB = B // 8          # batches per core
DECAY = 0.9
INHIB = 0.1

_CACHE = {}


def _register_dve_ops():
    import concourse.dve_ops as dve_ops
    from concourse.dve_spec import Spec, Src0, Src1, C1, select, minn, SubIdx
    from concourse.dve_table_gen import dve_ver_for

    if "ARF_PAIRFIRE" in dve_ops._SUB_OPCODE_FOR_NAME:
        from concourse.dve_ops import OPS
        by_name = {op.name: op for op in OPS}
        return by_name["ARF_PAIRFIRE"]

    # in0 = u (PSUM, page-broadcast), in1 = sigma = 10*thr (page-broadcast),
    # s1 (C1) = 0.1.  h = 0.1*sigma = thr.
    # page 0 (SubIdx falsy) -> spike = (u >= thr)
    # page 1                -> mem'  = min(u, u - thr)   (valid: thr > 0)
    h = Src1 * C1
    body = select(SubIdx, minn(Src0, Src0 - h), Src0 >= h)

    def ref(in0, in1, s0, s1, imm2):
        u = in0.astype(np.float32)
        thr = (in1 * np.float32(s1)).astype(np.float32)
        spk = (u >= thr).astype(np.float32)
        memf = np.minimum(u, (u - thr).astype(np.float32))
        pages = np.arange(in0.shape[1])[None, :, None]
        return np.where(pages == 0, spk, memf).astype(np.float32)

    spec = Spec(body=body, reference=ref)
    op = dve_ops.DveOp("ARF_PAIRFIRE", spec, subdim=True, uops_sha={})
    dve_ops.OPS.append(op)
    dve_ops.CUSTOM_DVE_SPECS["ARF_PAIRFIRE"] = spec
    dve_ops._SUB_OPCODE_FOR_NAME["ARF_PAIRFIRE"] = (
        dve_ops._CUSTOM_DVE_ROW_BASE + len(dve_ops.OPS) - 1
    )
    ver = dve_ver_for("TRN2")
    try:
        op.compile(ver)
    except ValueError as e:
        import re

        m = re.search(r":\s*([0-9a-f]{8,})\s*≠", str(e))
        if m is None:
            raise
        op.uops_sha[ver] = m.group(1)
        dve_ops._COMPILE_CACHE.pop(("ARF_PAIRFIRE", ver), None)
        op.compile(ver)
    return op


def _build(n_steps):
    import concourse.bass as bass
    from concourse import bacc
    import concourse.mybir as mybir
    from concourse import tile

    PAIRFIRE = _register_dve_ops()

    f32 = mybir.dt.float32
    f32r = mybir.dt.float32r
    Alu = mybir.AluOpType
    Act = mybir.ActivationFunctionType

    nc = bacc.Bacc()
    # xt[i, t*4+b4] = x[b4, t, i]  (host pre-transposed, per core)
    xt = nc.declare_dram_parameter("xt", [I, NB * n_steps], f32, isOutput=False)
    # cst: [w blocks (4x512) | br (4) | eyes (256)] -- single constants DMA
    # cst[:, k*512+o] = W[o, k*128+p]; cst[:, 2048+k] = bias[k*128+p]
    # cst[:, 2052:2180] = I; cst[:, 2180:2308] = 0.9*I
    cst = nc.declare_dram_parameter("cst", [128, 2308], f32, isOutput=False)
    # out[o_in, q*(n_steps*8) + t*8 + b2*4 + k] = spikes[q*2+b2, t, k*128+o_in]
    out_d = nc.declare_dram_parameter("out", [128, n_steps * 16], f32, isOutput=True)

    KB = I // 128            # 4 contraction blocks
    OB = O // 128            # 4 output blocks
    TCH = min(128, n_steps)  # timesteps per chunk
    assert n_steps % TCH == 0
    NCH = n_steps // TCH

    # per-chain mega-tile column layout: [OUT0 | MB0 | OUT1 | MB1]
    # (separate tile per chain so hazard intervals never cross chains)
    OUTW = TCH * 8
    MBW = (TCH + 1) * 8
    OUTC = [0, OUTW + MBW]
    MBC = [OUTW, 2 * OUTW + MBW]
    MEGA_W = 2 * (OUTW + MBW)
    DPAGE = MBC[0] - OUTC[0] + 8      # +8: mem slot t+1 vs spike slot t

    with tile.TileContext(nc) as tc:
        with (
            tc.tile_pool(name="const", bufs=1) as constp,
            tc.tile_pool(name="state", bufs=1) as statep,
            tc.tile_pool(name="big", bufs=1) as bigp,
            tc.tile_pool(name="xin", bufs=4) as xinp,
            tc.tile_pool(name="ps1", bufs=4, space=bass.MemorySpace.PSUM) as ps1p,
            tc.tile_pool(name="ps2", bufs=4, space=bass.MemorySpace.PSUM) as ps2p,
        ):
            # ---- constants (one DMA + ACT proxy copies so PE waits on one sem) ----
            c_raw = constp.tile([128, 2308], f32, tag="craw")
            w_sb = constp.tile([128, KB * O], f32, tag="w")
            br_sb = constp.tile([128, 4], f32, tag="br")
            ey_sb = constp.tile([128, 256], f32, tag="ey")
            ones_sb = constp.tile([128, 128], f32, tag="ones")
            warm = constp.tile([128, 1], f32, tag="warm")
            # tiny ACT op up front: act-table load overlaps the DMAs
            nc.vector.memset(warm[:], 0.0)
            nc.scalar.activation(warm[:], warm[:], Act.Identity, bias=0.0, scale=1.0)
            # per-k W DMA + copy thunks; interleaved with the x DMAs below so
            # the k=0 matmuls start after just two transfers
            def w_load(k):
                nc.gpsimd.dma_start(c_raw[:, k * O:(k + 1) * O],
                                    cst[:, k * O:(k + 1) * O])
                nc.scalar.copy(w_sb[:, k * O:(k + 1) * O].bitcast(f32r),
                               c_raw[:, k * O:(k + 1) * O])

            def emit_const_tail():
                nc.gpsimd.dma_start(c_raw[:, KB * O:], cst[:, KB * O:])
                nc.scalar.copy(br_sb[:], c_raw[:, KB * O:KB * O + 4])
                nc.scalar.copy(ey_sb[:], c_raw[:, KB * O + 4:KB * O + 260])

            nc.vector.memset(ones_sb[:], -INHIB)
            eye1 = ey_sb[:, 0:128]
            eye9 = ey_sb[:, 128:256]



            # ---- persistent buffers ----
            proj = bigp.tile([128, n_steps * 16], f32, tag="proj")
            megaA = bigp.tile([128, MEGA_W], f32, tag="megaA")
            megaB = bigp.tile([128, MEGA_W], f32, tag="megaB")
            mega = [megaA, megaB]
            proj4 = proj[:].rearrange("p (t b o) -> p t b o", b=NB, o=OB)

            sgA0 = statep.tile([128, 8], f32, tag="sgA0")
            sgA1 = statep.tile([128, 8], f32, tag="sgA1")
            sgB0 = statep.tile([128, 8], f32, tag="sgB0")
            sgB1 = statep.tile([128, 8], f32, tag="sgB1")
            sg = [[sgA0, sgA1], [sgB0, sgB1]]
            s09 = statep.tile([128, 8], f32, tag="s09")   # Pool scratch: 0.9*sigma
            for q in range(2):
                nc.vector.memset(sg[q][0][:], 10.0)       # sigma = 10*thr, thr0=1
                nc.vector.memset(mega[q][:, MBC[0]:MBC[0] + 8], 0.0)   # mem0 = 0

            def emit_phase1(ch, sink):
                """Emit phase-1 work units for chunk ch into sink (list of
                thunks); caller interleaves them between steps."""
                c0 = ch * TCH * NB
                xr = xinp.tile([128, KB * TCH * NB], f32, tag="xraw")
                xa = xinp.tile([128, KB * TCH * NB], f32, tag="x")
                xoff = [k * TCH * NB for k in range(KB)]
                for k in range(KB):
                    def dma(xr=xr, c0=c0, k=k):
                        nc.gpsimd.dma_start(
                            xr[:, xoff[k]:xoff[k] + TCH * NB],
                            xt[k * 128:(k + 1) * 128, c0:c0 + TCH * NB])
                    sink.append(dma)
                # 256-col matmul pieces so each fits a PE idle window between
                # recurrence steps without delaying them; pc-major order +
                # per-piece ACT copies so the chunk's first steps unblock
                # after ~1/NPC of the work
                PC = min(256, TCH * NB)
                NPC = (TCH * NB) // PC
                TPP = PC // NB           # timesteps covered per piece
                pss = []
                for ob in range(OB):
                    ps = ps1p.tile([128, TCH * NB], f32, tag="mmps")
                    pss.append(ps)
                for pc in range(NPC):
                    # fp32r-rounding proxy copies for this piece's x columns
                    for k in range(KB):
                        def xcp(xr=xr, xa=xa, xo=xoff[k], pc=pc):
                            nc.scalar.copy(
                                xa[:, xo + pc * PC: xo + (pc + 1) * PC].bitcast(f32r),
                                xr[:, xo + pc * PC: xo + (pc + 1) * PC])
                        sink.append(xcp)
                    for ob in range(OB):
                        ps = pss[ob]
                        for k in range(KB):
                            def mm(ps=ps, k=k, ob=ob, xo=xoff[k], xa=xa, pc=pc):
                                nc.tensor.matmul(
                                    ps[:, pc * PC:(pc + 1) * PC],
                                    w_sb[:, k * O + ob * 128: k * O + ob * 128 + 128].bitcast(f32r),
                                    xa[:, xo + pc * PC: xo + (pc + 1) * PC].bitcast(f32r),
                                    start=(k == 0),
                                    stop=(k == KB - 1),
                                )
                            sink.append(mm)
                        def cpy(ps=ps, ob=ob, ch=ch, pc=pc):
                            src = ps[:, pc * PC:(pc + 1) * PC].rearrange(
                                "p (t b) -> p t b", b=NB)
                            dst = proj4[:, ch * TCH + pc * TPP:
                                        ch * TCH + (pc + 1) * TPP, :, ob]
                            nc.scalar.activation(
                                dst, src, Act.Identity,
                                bias=br_sb[:, ob:ob + 1], scale=1.0,
                            )
                        sink.append(cpy)

            # chunk 0 phase 1 runs up front; W and x transfers interleaved,
            # the small br/eyes constants last (not needed until step 0)
            pre = []
            emit_phase1(0, pre)
            for k in range(KB):
                w_load(k)
                pre[k]()                # x_k DMA
            emit_const_tail()
            for f in pre[KB:]:
                f()

            # ---- main loop ----
            for ch in range(NCH):
                par = ch % 2
                outc = OUTC[par]
                mbc = MBC[par]
                # next chunk's phase-1, interleaved between steps
                nxt = []
                if ch + 1 < NCH:
                    emit_phase1(ch + 1, nxt)
                nxt_per_step = -(-len(nxt) // TCH) if nxt else 0

                for tl in range(TCH):
                    t = ch * TCH + tl
                    # source column of mem state
                    if tl == 0:
                        mcol = MBC[(ch - 1) % 2] + TCH * 8 if ch > 0 else MBC[0]
                    else:
                        mcol = mbc + tl * 8
                    for q in range(2):
                        mg = mega[q]
                        s_cur = sg[q][t % 2]
                        ps = ps2p.tile([128, 8], f32, tag="u")
                        qc = q * 8
                        # u = proj[t] (+psum) ...
                        nc.tensor.matmul(
                            ps[:], eye1, proj[:, t * 16 + qc: t * 16 + qc + 8],
                            start=True, stop=False,
                        )
                        # ... - 0.1 * sum_o(mem)  (4 o-block reduce matmuls,
                        #     col (b2,k) read dup x4 via stride tricks)
                        for k in range(OB):
                            mv = mg[:, mcol + k: mcol + k + 5: 4]
                            mv = mv.rearrange("p (b x) -> p b x", x=1).broadcast_to((128, 2, 4))
                            nc.tensor.matmul(ps[:], ones_sb[:], mv,
                                             start=False, stop=False)
                        # ... + 0.9 * mem
                        nc.tensor.matmul(
                            ps[:], eye9, mg[:, mcol: mcol + 8],
                            start=False, stop=True,
                        )
                        # pages: 0 -> spike @ OUT slot tl, 1 -> mem' @ MB slot tl+1
                        outap = mg[:, outc + tl * 8: outc + tl * 8 + 8]
                        outap = outap.rearrange("p (x c) -> p x c", x=1).copy()
                        outap.ap[1] = (DPAGE, 2)
                        in0 = ps[:].rearrange("p (x c) -> p x c", x=1).broadcast_to((128, 2, 8))
                        in1 = s_cur[:].rearrange("p (x c) -> p x c", x=1).broadcast_to((128, 2, 8))
                        nc.vector._custom_dve(
                            PAIRFIRE, out=outap, in0=in0, in1=in1, s1=INHIB,
                        )
                    # sigma' = 0.9*sigma + spk (DVE stt: same-engine in-order
                    # after both pair ops -> zero sem cost, hides in latency)
                    for q in range(2):
                        spk = mega[q][:, outc + tl * 8: outc + tl * 8 + 8]
                        nc.vector.scalar_tensor_tensor(
                            sg[q][(t + 1) % 2][:], sg[q][t % 2][:], DECAY,
                            spk, Alu.mult, Alu.add,
                        )
                    # interleave next chunk's phase-1 work
                    for _ in range(nxt_per_step):
                        if nxt:
                            nxt.pop(0)()
                    # stream this chunk's spikes out early in pieces (the
                    # last chunk in quarters to shrink the end-of-kernel tail)
                    nq = 4 if ch == NCH - 1 else 2
                    if TCH >= nq and (tl + 1) % (TCH // nq) == 0 and tl + 1 < TCH:
                        piece = OUTW // nq
                        lo = ((tl + 1) // (TCH // nq) - 1) * piece
                        for q in range(2):
                            nc.gpsimd.dma_start(
                                out_d[:, q * n_steps * 8 + ch * OUTW + lo:
                                      q * n_steps * 8 + ch * OUTW + lo + piece],
                                mega[q][:, outc + lo:outc + lo + piece],
                            )
                for f in nxt:
                    f()
                # stream the rest of this chunk's spikes out
                nq = 4 if ch == NCH - 1 else 2
                h = OUTW - OUTW // nq if TCH >= nq else 0
                for q in range(2):
                    nc.gpsimd.dma_start(
                        out_d[:, q * n_steps * 8 + ch * OUTW + h:
                              q * n_steps * 8 + (ch + 1) * OUTW],
                        mega[q][:, outc + h:outc + OUTW],
                    )

    nc.compile()
    return nc


def _get_nc(n_steps):
    if n_steps not in _CACHE:
        _CACHE[n_steps] = _build(n_steps)
    return _CACHE[n_steps]


def _host_pack(x, W, b, n_steps):
    xs_all = []
    for c in range(8):
        xs = x[c * NB:(c + 1) * NB, :n_steps, :]          # [NB, S, I]
        xtc = np.ascontiguousarray(xs.transpose(2, 1, 0)).reshape(I, n_steps * NB)
        xs_all.append(xtc)
    wt = np.ascontiguousarray(W.T)                         # [I, O]
    wblk = np.concatenate([wt[k * 128:(k + 1) * 128] for k in range(4)], axis=1)
    br = np.ascontiguousarray(b.reshape(4, 128).T)         # [128, 4]
    eye = np.eye(128, dtype=np.float32)
    cst = np.concatenate(
        [wblk, br, eye, (np.float32(DECAY) * eye)], axis=1
    ).astype(np.float32)                                   # [128, 2308]
    return [{"xt": xs_all[c], "cst": cst} for c in range(8)]


def _host_unpack(outs, n_steps):
    full = np.empty((B, n_steps, O), dtype=np.float32)
    for c in range(8):
        o = outs[c]["out"].reshape(128, 2, n_steps, 2, 4)  # [o_in, q, t, b2, k]
        full[c * NB:(c + 1) * NB] = np.ascontiguousarray(
            o.transpose(1, 3, 2, 4, 0)).reshape(NB, n_steps, O)
    return full


def kernel(x, W, b, n_steps=S, trace=False):
    from concourse.bass_utils import run_bass_kernel_spmd

    x = np.asarray(x, dtype=np.float32)
    W = np.asarray(W, dtype=np.float32)
    b = np.asarray(b, dtype=np.float32)
    nc = _get_nc(n_steps)
    in_maps = _host_pack(x, W, b, n_steps)
    res = run_bass_kernel_spmd(nc, in_maps, core_ids=list(range(8)), trace=trace)
    out = _host_unpack(res.results, n_steps)
    kernel.last_result = res
    return out


# revision 33
# speedup vs baseline: 1.0066x; 1.0008x over previous
"""AdaptiveResonateAndFireNeuron Trainium2 kernel v2 (8 cores, data-parallel batch).

Per batch row b:
  proj[t] = x[b,t,:] @ W.T + bias
  u       = 0.9*mem + proj[t] - 0.1*sum_o(mem)
  spk     = (u >= thr);  mem' = u - spk*thr;  thr' = 0.9*thr + 0.1*spk

v2 structure (vs v1 baseline):
- phase 1 (proj) matmuls in float32r (1 cyc/row at 512 cols vs 4 for fp32);
  PSUM->SBUF copy + bias moved from DVE to the Activation engine.
- phase 2: u is accumulated ENTIRELY in PSUM by 6 matmuls per step per chain
  (identity@proj + 4x (-0.1*ones)@mem [o-block reduce] + 0.9*identity@mem).
  A single custom DVE op (ARF_PAIRFIRE, subdim pages) then reads PSUM once and
  writes BOTH outputs: page0 = spike (straight into the DMA staging buffer),
  page1 = mem' (into the mem trajectory ring).  Threshold state is kept scaled
  (sigma = 10*thr) and updated on the GpSimd/Pool engine with ONE
  scalar_tensor_tensor: sigma' = 0.9*sigma + spk.
- two independent chains (2 batches each) interleave so the PE->DVE->PE
  latency of one chain hides under the other.

Core layout: partition p = o_in (o % 128); per-step col c = b4*4 + k where
b4 = local batch (0..3), k = o_blk (o // 128); chain q = b4 // 2.
"""

import numpy as np

B, S, I, O = 32, 1024, 512, 512

SIGMA_ENGINE = "gpsimd"          # "gpsimd" (Pool) or "vector" (DVE)


def _SIGMA_ENG(nc):
    return getattr(nc, SIGMA_ENGINE)
NB = B // 8          # batches per core
DECAY = 0.9
INHIB = 0.1

_CACHE = {}


def _register_dve_ops():
    import concourse.dve_ops as dve_ops
    from concourse.dve_spec import Spec, Src0, Src1, C1, select, minn, SubIdx
    from concourse.dve_table_gen import dve_ver_for

    if "ARF_PAIRFIRE" in dve_ops._SUB_OPCODE_FOR_NAME:
        from concourse.dve_ops import OPS
        by_name = {op.name: op for op in OPS}
        return by_name["ARF_PAIRFIRE"]

    # in0 = u (PSUM, page-broadcast), in1 = sigma = 10*thr (page-broadcast),
    # s1 (C1) = 0.1.  h = 0.1*sigma = thr.
    # page 0 (SubIdx falsy) -> spike = (u >= thr)
    # page 1                -> mem'  = min(u, u - thr)   (valid: thr > 0)
    h = Src1 * C1
    body = select(SubIdx, minn(Src0, Src0 - h), Src0 >= h)

    def ref(in0, in1, s0, s1, imm2):
        u = in0.astype(np.float32)
        thr = (in1 * np.float32(s1)).astype(np.float32)
        spk = (u >= thr).astype(np.float32)
        memf = np.minimum(u, (u - thr).astype(np.float32))
        pages = np.arange(in0.shape[1])[None, :, None]
        return np.where(pages == 0, spk, memf).astype(np.float32)

    spec = Spec(body=body, reference=ref)
    op = dve_ops.DveOp("ARF_PAIRFIRE", spec, subdim=True, uops_sha={})
    dve_ops.OPS.append(op)
    dve_ops.CUSTOM_DVE_SPECS["ARF_PAIRFIRE"] = spec
    dve_ops._SUB_OPCODE_FOR_NAME["ARF_PAIRFIRE"] = (
        dve_ops._CUSTOM_DVE_ROW_BASE + len(dve_ops.OPS) - 1
    )
    ver = dve_ver_for("TRN2")
    try:
        op.compile(ver)
    except ValueError as e:
        import re

        m = re.search(r":\s*([0-9a-f]{8,})\s*≠", str(e))
        if m is None:
            raise
        op.uops_sha[ver] = m.group(1)
        dve_ops._COMPILE_CACHE.pop(("ARF_PAIRFIRE", ver), None)
        op.compile(ver)
    return op


def _build(n_steps):
    import concourse.bass as bass
    from concourse import bacc
    import concourse.mybir as mybir
    from concourse import tile

    PAIRFIRE = _register_dve_ops()

    f32 = mybir.dt.float32
    f32r = mybir.dt.float32r
    Alu = mybir.AluOpType
    Act = mybir.ActivationFunctionType

    nc = bacc.Bacc()
    # xt[i, t*4+b4] = x[b4, t, i]  (host pre-transposed, per core)
    xt = nc.declare_dram_parameter("xt", [I, NB * n_steps], f32, isOutput=False)
    # cst: [w blocks (4x512) | br (4) | eyes (256)] -- single constants DMA
    # cst[:, k*512+o] = W[o, k*128+p]; cst[:, 2048+k] = bias[k*128+p]
    # cst[:, 2052:2180] = I; cst[:, 2180:2308] = 0.9*I
    cst = nc.declare_dram_parameter("cst", [128, 2308], f32, isOutput=False)
    # out[o_in, q*(n_steps*8) + t*8 + b2*4 + k] = spikes[q*2+b2, t, k*128+o_in]
    out_d = nc.declare_dram_parameter("out", [128, n_steps * 16], f32, isOutput=True)

    KB = I // 128            # 4 contraction blocks
    OB = O // 128            # 4 output blocks
    TCH = min(128, n_steps)  # timesteps per chunk
    assert n_steps % TCH == 0
    NCH = n_steps // TCH

    # per-chain mega-tile column layout: [OUT0 | MB0 | OUT1 | MB1]
    # (separate tile per chain so hazard intervals never cross chains)
    OUTW = TCH * 8
    MBW = (TCH + 1) * 8
    OUTC = [0, OUTW + MBW]
    MBC = [OUTW, 2 * OUTW + MBW]
    MEGA_W = 2 * (OUTW + MBW)
    DPAGE = MBC[0] - OUTC[0] + 8      # +8: mem slot t+1 vs spike slot t

    with tile.TileContext(nc) as tc:
        with (
            tc.tile_pool(name="const", bufs=1) as constp,
            tc.tile_pool(name="state", bufs=1) as statep,
            tc.tile_pool(name="big", bufs=1) as bigp,
            tc.tile_pool(name="xin", bufs=4) as xinp,
            tc.tile_pool(name="ps1", bufs=4, space=bass.MemorySpace.PSUM) as ps1p,
            tc.tile_pool(name="ps2", bufs=4, space=bass.MemorySpace.PSUM) as ps2p,
        ):
            # ---- constants (one DMA + ACT proxy copies so PE waits on one sem) ----
            c_raw = constp.tile([128, 2308], f32, tag="craw")
            w_sb = constp.tile([128, KB * O], f32, tag="w")
            br_sb = constp.tile([128, 4], f32, tag="br")
            ey_sb = constp.tile([128, 256], f32, tag="ey")
            ones_sb = constp.tile([128, 128], f32, tag="ones")
            warm = constp.tile([128, 1], f32, tag="warm")
            # tiny ACT op up front: act-table load overlaps the DMAs
            nc.vector.memset(warm[:], 0.0)
            nc.scalar.activation(warm[:], warm[:], Act.Identity, bias=0.0, scale=1.0)
            # per-k W DMA + copy thunks; interleaved with the x DMAs below so
            # the k=0 matmuls start after just two transfers
            def w_load(k):
                nc.gpsimd.dma_start(c_raw[:, k * O:(k + 1) * O],
                                    cst[:, k * O:(k + 1) * O])
                nc.scalar.copy(w_sb[:, k * O:(k + 1) * O].bitcast(f32r),
                               c_raw[:, k * O:(k + 1) * O])

            def emit_const_tail():
                nc.gpsimd.dma_start(c_raw[:, KB * O:], cst[:, KB * O:])
                nc.scalar.copy(br_sb[:], c_raw[:, KB * O:KB * O + 4])
                nc.scalar.copy(ey_sb[:], c_raw[:, KB * O + 4:KB * O + 260])

            nc.vector.memset(ones_sb[:], -INHIB)
            eye1 = ey_sb[:, 0:128]
            eye9 = ey_sb[:, 128:256]



            # ---- persistent buffers ----
            proj = bigp.tile([128, n_steps * 16], f32, tag="proj")
            megaA = bigp.tile([128, MEGA_W], f32, tag="megaA")
            megaB = bigp.tile([128, MEGA_W], f32, tag="megaB")
            mega = [megaA, megaB]
            proj4 = proj[:].rearrange("p (t b o) -> p t b o", b=NB, o=OB)

            sgA0 = statep.tile([128, 8], f32, tag="sgA0")
            sgA1 = statep.tile([128, 8], f32, tag="sgA1")
            sgB0 = statep.tile([128, 8], f32, tag="sgB0")
            sgB1 = statep.tile([128, 8], f32, tag="sgB1")
            sg = [[sgA0, sgA1], [sgB0, sgB1]]
            s09 = statep.tile([128, 8], f32, tag="s09")   # Pool scratch: 0.9*sigma
            for q in range(2):
                nc.vector.memset(sg[q][0][:], 10.0)       # sigma = 10*thr, thr0=1
                nc.vector.memset(mega[q][:, MBC[0]:MBC[0] + 8], 0.0)   # mem0 = 0

            def emit_phase1(ch, sink):
                """Emit phase-1 work units for chunk ch into sink (list of
                thunks); caller interleaves them between steps."""
                c0 = ch * TCH * NB
                xr = xinp.tile([128, KB * TCH * NB], f32, tag="xraw")
                xa = xinp.tile([128, KB * TCH * NB], f32, tag="x")
                xoff = [k * TCH * NB for k in range(KB)]
                for k in range(KB):
                    def dma(xr=xr, c0=c0, k=k):
                        nc.gpsimd.dma_start(
                            xr[:, xoff[k]:xoff[k] + TCH * NB],
                            xt[k * 128:(k + 1) * 128, c0:c0 + TCH * NB])
                    sink.append(dma)
                # 256-col matmul pieces so each fits a PE idle window between
                # recurrence steps without delaying them; pc-major order +
                # per-piece ACT copies so the chunk's first steps unblock
                # after ~1/NPC of the work
                PC = min(256, TCH * NB)
                NPC = (TCH * NB) // PC
                TPP = PC // NB           # timesteps covered per piece
                pss = []
                for ob in range(OB):
                    ps = ps1p.tile([128, TCH * NB], f32, tag="mmps")
                    pss.append(ps)
                for pc in range(NPC):
                    # fp32r-rounding proxy copies for this piece's x columns
                    for k in range(KB):
                        def xcp(xr=xr, xa=xa, xo=xoff[k], pc=pc):
                            nc.scalar.copy(
                                xa[:, xo + pc * PC: xo + (pc + 1) * PC].bitcast(f32r),
                                xr[:, xo + pc * PC: xo + (pc + 1) * PC])
                        sink.append(xcp)
                    for ob in range(OB):
                        ps = pss[ob]
                        for k in range(KB):
                            def mm(ps=ps, k=k, ob=ob, xo=xoff[k], xa=xa, pc=pc):
                                nc.tensor.matmul(
                                    ps[:, pc * PC:(pc + 1) * PC],
                                    w_sb[:, k * O + ob * 128: k * O + ob * 128 + 128].bitcast(f32r),
                                    xa[:, xo + pc * PC: xo + (pc + 1) * PC].bitcast(f32r),
                                    start=(k == 0),
                                    stop=(k == KB - 1),
                                )
                            sink.append(mm)
                        def cpy(ps=ps, ob=ob, ch=ch, pc=pc):
                            src = ps[:, pc * PC:(pc + 1) * PC].rearrange(
                                "p (t b) -> p t b", b=NB)
                            dst = proj4[:, ch * TCH + pc * TPP:
                                        ch * TCH + (pc + 1) * TPP, :, ob]
                            nc.scalar.activation(
                                dst, src, Act.Identity,
                                bias=br_sb[:, ob:ob + 1], scale=1.0,
                            )
                        sink.append(cpy)

            # chunk 0 phase 1 runs up front; W and x transfers interleaved,
            # the small br/eyes constants last (not needed until step 0)
            pre = []
            emit_phase1(0, pre)
            for k in range(KB):
                w_load(k)
                pre[k]()                # x_k DMA
            emit_const_tail()
            for f in pre[KB:]:
                f()

            # ---- main loop ----
            for ch in range(NCH):
                par = ch % 2
                outc = OUTC[par]
                mbc = MBC[par]
                # next chunk's phase-1, interleaved between steps
                nxt = []
                if ch + 1 < NCH:
                    emit_phase1(ch + 1, nxt)
                nxt_per_step = -(-len(nxt) // TCH) if nxt else 0

                for tl in range(TCH):
                    t = ch * TCH + tl
                    # source column of mem state
                    if tl == 0:
                        mcol = MBC[(ch - 1) % 2] + TCH * 8 if ch > 0 else MBC[0]
                    else:
                        mcol = mbc + tl * 8
                    for q in range(2):
                        mg = mega[q]
                        s_cur = sg[q][t % 2]
                        ps = ps2p.tile([128, 8], f32, tag="u")
                        qc = q * 8
                        # u = proj[t] (+psum) ...
                        nc.tensor.matmul(
                            ps[:], eye1, proj[:, t * 16 + qc: t * 16 + qc + 8],
                            start=True, stop=False,
                        )
                        # ... - 0.1 * sum_o(mem)  (4 o-block reduce matmuls,
                        #     col (b2,k) read dup x4 via stride tricks)
                        for k in range(OB):
                            mv = mg[:, mcol + k: mcol + k + 5: 4]
                            mv = mv.rearrange("p (b x) -> p b x", x=1).broadcast_to((128, 2, 4))
                            nc.tensor.matmul(ps[:], ones_sb[:], mv,
                                             start=False, stop=False)
                        # ... + 0.9 * mem
                        nc.tensor.matmul(
                            ps[:], eye9, mg[:, mcol: mcol + 8],
                            start=False, stop=True,
                        )
                        # pages: 0 -> spike @ OUT slot tl, 1 -> mem' @ MB slot tl+1
                        outap = mg[:, outc + tl * 8: outc + tl * 8 + 8]
                        outap = outap.rearrange("p (x c) -> p x c", x=1).copy()
                        outap.ap[1] = (DPAGE, 2)
                        in0 = ps[:].rearrange("p (x c) -> p x c", x=1).broadcast_to((128, 2, 8))
                        in1 = s_cur[:].rearrange("p (x c) -> p x c", x=1).broadcast_to((128, 2, 8))
                        nc.vector._custom_dve(
                            PAIRFIRE, out=outap, in0=in0, in1=in1, s1=INHIB,
                        )
                    # sigma' = 0.9*sigma + spk (DVE stt: same-engine in-order
                    # after both pair ops -> zero sem cost, hides in latency)
                    for q in range(2):
                        spk = mega[q][:, outc + tl * 8: outc + tl * 8 + 8]
                        nc.vector.scalar_tensor_tensor(
                            sg[q][(t + 1) % 2][:], sg[q][t % 2][:], DECAY,
                            spk, Alu.mult, Alu.add,
                        )
                    # interleave next chunk's phase-1 work
                    for _ in range(nxt_per_step):
                        if nxt:
                            nxt.pop(0)()
                    # stream this chunk's spikes out early in pieces (the
                    # last chunk in quarters to shrink the end-of-kernel tail)
                    nq = 4 if ch == NCH - 1 else 2
                    if TCH >= nq and (tl + 1) % (TCH // nq) == 0 and tl + 1 < TCH:
                        piece = OUTW // nq
                        lo = ((tl + 1) // (TCH // nq) - 1) * piece
                        for q in range(2):
                            nc.scalar.dma_start(
                                out_d[:, q * n_steps * 8 + ch * OUTW + lo:
                                      q * n_steps * 8 + ch * OUTW + lo + piece],
                                mega[q][:, outc + lo:outc + lo + piece],
                            )
                for f in nxt:
                    f()
                # stream the rest of this chunk's spikes out
                nq = 4 if ch == NCH - 1 else 2
                h = OUTW - OUTW // nq if TCH >= nq else 0
                for q in range(2):
                    nc.scalar.dma_start(
                        out_d[:, q * n_steps * 8 + ch * OUTW + h:
                              q * n_steps * 8 + (ch + 1) * OUTW],
                        mega[q][:, outc + h:outc + OUTW],
                    )

    nc.compile()
    return nc


def _get_nc(n_steps):
    if n_steps not in _CACHE:
        _CACHE[n_steps] = _build(n_steps)
    return _CACHE[n_steps]


def _host_pack(x, W, b, n_steps):
    xs_all = []
    for c in range(8):
        xs = x[c * NB:(c + 1) * NB, :n_steps, :]          # [NB, S, I]
        xtc = np.ascontiguousarray(xs.transpose(2, 1, 0)).reshape(I, n_steps * NB)
        xs_all.append(xtc)
    wt = np.ascontiguousarray(W.T)                         # [I, O]
    wblk = np.concatenate([wt[k * 128:(k + 1) * 128] for k in range(4)], axis=1)
    br = np.ascontiguousarray(b.reshape(4, 128).T)         # [128, 4]
    eye = np.eye(128, dtype=np.float32)
    cst = np.concatenate(
        [wblk, br, eye, (np.float32(DECAY) * eye)], axis=1
    ).astype(np.float32)                                   # [128, 2308]
    return [{"xt": xs_all[c], "cst": cst} for c in range(8)]


def _host_unpack(outs, n_steps):
    full = np.empty((B, n_steps, O), dtype=np.float32)
    for c in range(8):
        o = outs[c]["out"].reshape(128, 2, n_steps, 2, 4)  # [o_in, q, t, b2, k]
        full[c * NB:(c + 1) * NB] = np.ascontiguousarray(
            o.transpose(1, 3, 2, 4, 0)).reshape(NB, n_steps, O)
    return full


def kernel(x, W, b, n_steps=S, trace=False):
    from concourse.bass_utils import run_bass_kernel_spmd

    x = np.asarray(x, dtype=np.float32)
    W = np.asarray(W, dtype=np.float32)
    b = np.asarray(b, dtype=np.float32)
    nc = _get_nc(n_steps)
    in_maps = _host_pack(x, W, b, n_steps)
    res = run_bass_kernel_spmd(nc, in_maps, core_ids=list(range(8)), trace=trace)
    out = _host_unpack(res.results, n_steps)
    kernel.last_result = res
    return out


# revision 36
# speedup vs baseline: 1.0072x; 1.0006x over previous
"""AdaptiveResonateAndFireNeuron Trainium2 kernel v2 (8 cores, data-parallel batch).

Per batch row b:
  proj[t] = x[b,t,:] @ W.T + bias
  u       = 0.9*mem + proj[t] - 0.1*sum_o(mem)
  spk     = (u >= thr);  mem' = u - spk*thr;  thr' = 0.9*thr + 0.1*spk

v2 structure (vs v1 baseline):
- phase 1 (proj) matmuls in float32r (1 cyc/row at 512 cols vs 4 for fp32);
  PSUM->SBUF copy + bias moved from DVE to the Activation engine.
- phase 2: u is accumulated ENTIRELY in PSUM by 6 matmuls per step per chain
  (identity@proj + 4x (-0.1*ones)@mem [o-block reduce] + 0.9*identity@mem).
  A single custom DVE op (ARF_PAIRFIRE, subdim pages) then reads PSUM once and
  writes BOTH outputs: page0 = spike (straight into the DMA staging buffer),
  page1 = mem' (into the mem trajectory ring).  Threshold state is kept scaled
  (sigma = 10*thr) and updated on the GpSimd/Pool engine with ONE
  scalar_tensor_tensor: sigma' = 0.9*sigma + spk.
- two independent chains (2 batches each) interleave so the PE->DVE->PE
  latency of one chain hides under the other.

Core layout: partition p = o_in (o % 128); per-step col c = b4*4 + k where
b4 = local batch (0..3), k = o_blk (o // 128); chain q = b4 // 2.
"""

import numpy as np

B, S, I, O = 32, 1024, 512, 512

SIGMA_ENGINE = "gpsimd"          # "gpsimd" (Pool) or "vector" (DVE)


def _SIGMA_ENG(nc):
    return getattr(nc, SIGMA_ENGINE)
NB = B // 8          # batches per core
DECAY = 0.9
INHIB = 0.1

_CACHE = {}


def _register_dve_ops():
    import concourse.dve_ops as dve_ops
    from concourse.dve_spec import Spec, Src0, Src1, C1, select, minn, SubIdx
    from concourse.dve_table_gen import dve_ver_for

    if "ARF_PAIRFIRE" in dve_ops._SUB_OPCODE_FOR_NAME:
        from concourse.dve_ops import OPS
        by_name = {op.name: op for op in OPS}
        return by_name["ARF_PAIRFIRE"]

    # in0 = u (PSUM, page-broadcast), in1 = sigma = 10*thr (page-broadcast),
    # s1 (C1) = 0.1.  h = 0.1*sigma = thr.
    # page 0 (SubIdx falsy) -> spike = (u >= thr)
    # page 1                -> mem'  = min(u, u - thr)   (valid: thr > 0)
    h = Src1 * C1
    body = select(SubIdx, minn(Src0, Src0 - h), Src0 >= h)

    def ref(in0, in1, s0, s1, imm2):
        u = in0.astype(np.float32)
        thr = (in1 * np.float32(s1)).astype(np.float32)
        spk = (u >= thr).astype(np.float32)
        memf = np.minimum(u, (u - thr).astype(np.float32))
        pages = np.arange(in0.shape[1])[None, :, None]
        return np.where(pages == 0, spk, memf).astype(np.float32)

    spec = Spec(body=body, reference=ref)
    op = dve_ops.DveOp("ARF_PAIRFIRE", spec, subdim=True, uops_sha={})
    dve_ops.OPS.append(op)
    dve_ops.CUSTOM_DVE_SPECS["ARF_PAIRFIRE"] = spec
    dve_ops._SUB_OPCODE_FOR_NAME["ARF_PAIRFIRE"] = (
        dve_ops._CUSTOM_DVE_ROW_BASE + len(dve_ops.OPS) - 1
    )
    ver = dve_ver_for("TRN2")
    try:
        op.compile(ver)
    except ValueError as e:
        import re

        m = re.search(r":\s*([0-9a-f]{8,})\s*≠", str(e))
        if m is None:
            raise
        op.uops_sha[ver] = m.group(1)
        dve_ops._COMPILE_CACHE.pop(("ARF_PAIRFIRE", ver), None)
        op.compile(ver)
    return op


def _build(n_steps):
    import concourse.bass as bass
    from concourse import bacc
    import concourse.mybir as mybir
    from concourse import tile

    PAIRFIRE = _register_dve_ops()

    f32 = mybir.dt.float32
    f32r = mybir.dt.float32r
    Alu = mybir.AluOpType
    Act = mybir.ActivationFunctionType

    nc = bacc.Bacc()
    # xt[i, t*4+b4] = x[b4, t, i]  (host pre-transposed, per core)
    xt = nc.declare_dram_parameter("xt", [I, NB * n_steps], f32, isOutput=False)
    # cst: [w blocks (4x512) | br (4) | eyes (256)] -- single constants DMA
    # cst[:, k*512+o] = W[o, k*128+p]; cst[:, 2048+k] = bias[k*128+p]
    # cst[:, 2052:2180] = I; cst[:, 2180:2308] = 0.9*I
    cst = nc.declare_dram_parameter("cst", [128, 2308], f32, isOutput=False)
    # out[o_in, q*(n_steps*8) + t*8 + b2*4 + k] = spikes[q*2+b2, t, k*128+o_in]
    out_d = nc.declare_dram_parameter("out", [128, n_steps * 16], f32, isOutput=True)

    KB = I // 128            # 4 contraction blocks
    OB = O // 128            # 4 output blocks
    TCH = min(128, n_steps)  # timesteps per chunk
    assert n_steps % TCH == 0
    NCH = n_steps // TCH

    # per-chain mega-tile column layout: [OUT0 | MB0 | OUT1 | MB1]
    # (separate tile per chain so hazard intervals never cross chains)
    OUTW = TCH * 8
    MBW = (TCH + 1) * 8
    OUTC = [0, OUTW + MBW]
    MBC = [OUTW, 2 * OUTW + MBW]
    MEGA_W = 2 * (OUTW + MBW)
    DPAGE = MBC[0] - OUTC[0] + 8      # +8: mem slot t+1 vs spike slot t

    with tile.TileContext(nc) as tc:
        with (
            tc.tile_pool(name="const", bufs=1) as constp,
            tc.tile_pool(name="state", bufs=1) as statep,
            tc.tile_pool(name="big", bufs=1) as bigp,
            tc.tile_pool(name="xin", bufs=4) as xinp,
            tc.tile_pool(name="ps1", bufs=4, space=bass.MemorySpace.PSUM) as ps1p,
            tc.tile_pool(name="ps2", bufs=4, space=bass.MemorySpace.PSUM) as ps2p,
        ):
            # ---- constants (one DMA + ACT proxy copies so PE waits on one sem) ----
            c_raw = constp.tile([128, 2308], f32, tag="craw")
            w_sb = constp.tile([128, KB * O], f32, tag="w")
            br_sb = constp.tile([128, 4], f32, tag="br")
            ey_sb = constp.tile([128, 256], f32, tag="ey")
            ones_sb = constp.tile([128, 128], f32, tag="ones")
            warm = constp.tile([128, 1], f32, tag="warm")
            # tiny ACT op up front: act-table load overlaps the DMAs
            nc.vector.memset(warm[:], 0.0)
            nc.scalar.activation(warm[:], warm[:], Act.Identity, bias=0.0, scale=1.0)
            # per-k W DMA + copy thunks; interleaved with the x DMAs below so
            # the k=0 matmuls start after just two transfers
            def w_load(k):
                nc.gpsimd.dma_start(c_raw[:, k * O:(k + 1) * O],
                                    cst[:, k * O:(k + 1) * O])
                nc.scalar.copy(w_sb[:, k * O:(k + 1) * O].bitcast(f32r),
                               c_raw[:, k * O:(k + 1) * O])

            def emit_const_tail():
                nc.gpsimd.dma_start(c_raw[:, KB * O:], cst[:, KB * O:])
                nc.scalar.copy(br_sb[:], c_raw[:, KB * O:KB * O + 4])
                nc.scalar.copy(ey_sb[:], c_raw[:, KB * O + 4:KB * O + 260])

            nc.vector.memset(ones_sb[:], -INHIB)
            eye1 = ey_sb[:, 0:128]
            eye9 = ey_sb[:, 128:256]



            # ---- persistent buffers ----
            proj = bigp.tile([128, n_steps * 16], f32, tag="proj")
            megaA = bigp.tile([128, MEGA_W], f32, tag="megaA")
            megaB = bigp.tile([128, MEGA_W], f32, tag="megaB")
            mega = [megaA, megaB]
            proj4 = proj[:].rearrange("p (t b o) -> p t b o", b=NB, o=OB)

            sgA0 = statep.tile([128, 8], f32, tag="sgA0")
            sgA1 = statep.tile([128, 8], f32, tag="sgA1")
            sgB0 = statep.tile([128, 8], f32, tag="sgB0")
            sgB1 = statep.tile([128, 8], f32, tag="sgB1")
            sg = [[sgA0, sgA1], [sgB0, sgB1]]
            s09 = statep.tile([128, 8], f32, tag="s09")   # Pool scratch: 0.9*sigma
            for q in range(2):
                nc.vector.memset(sg[q][0][:], 10.0)       # sigma = 10*thr, thr0=1
                nc.vector.memset(mega[q][:, MBC[0]:MBC[0] + 8], 0.0)   # mem0 = 0

            def emit_phase1(ch, sink, sink2=None):
                """Emit phase-1 work units for chunk ch into sink (list of
                thunks); caller interleaves them between steps.  If sink2 is
                given, pieces pc>=1 go there instead (lets chunk 0's later
                pieces interleave with its first steps)."""
                c0 = ch * TCH * NB
                xr = xinp.tile([128, KB * TCH * NB], f32, tag="xraw")
                xa = xinp.tile([128, KB * TCH * NB], f32, tag="x")
                xoff = [k * TCH * NB for k in range(KB)]
                for k in range(KB):
                    def dma(xr=xr, c0=c0, k=k):
                        nc.gpsimd.dma_start(
                            xr[:, xoff[k]:xoff[k] + TCH * NB],
                            xt[k * 128:(k + 1) * 128, c0:c0 + TCH * NB])
                    sink.append(dma)
                # 256-col matmul pieces so each fits a PE idle window between
                # recurrence steps without delaying them; pc-major order +
                # per-piece ACT copies so the chunk's first steps unblock
                # after ~1/NPC of the work
                PC = min(256, TCH * NB)
                NPC = (TCH * NB) // PC
                TPP = PC // NB           # timesteps covered per piece
                pss = []
                for ob in range(OB):
                    ps = ps1p.tile([128, TCH * NB], f32, tag="mmps")
                    pss.append(ps)
                for pc in range(NPC):
                    tgt = sink if (pc == 0 or sink2 is None) else sink2
                    # fp32r-rounding proxy copies for this piece's x columns
                    for k in range(KB):
                        def xcp(xr=xr, xa=xa, xo=xoff[k], pc=pc):
                            nc.scalar.copy(
                                xa[:, xo + pc * PC: xo + (pc + 1) * PC].bitcast(f32r),
                                xr[:, xo + pc * PC: xo + (pc + 1) * PC])
                        tgt.append(xcp)
                    for ob in range(OB):
                        ps = pss[ob]
                        for k in range(KB):
                            def mm(ps=ps, k=k, ob=ob, xo=xoff[k], xa=xa, pc=pc):
                                nc.tensor.matmul(
                                    ps[:, pc * PC:(pc + 1) * PC],
                                    w_sb[:, k * O + ob * 128: k * O + ob * 128 + 128].bitcast(f32r),
                                    xa[:, xo + pc * PC: xo + (pc + 1) * PC].bitcast(f32r),
                                    start=(k == 0),
                                    stop=(k == KB - 1),
                                )
                            tgt.append(mm)
                        def cpy(ps=ps, ob=ob, ch=ch, pc=pc):
                            src = ps[:, pc * PC:(pc + 1) * PC].rearrange(
                                "p (t b) -> p t b", b=NB)
                            dst = proj4[:, ch * TCH + pc * TPP:
                                        ch * TCH + (pc + 1) * TPP, :, ob]
                            nc.scalar.activation(
                                dst, src, Act.Identity,
                                bias=br_sb[:, ob:ob + 1], scale=1.0,
                            )
                        tgt.append(cpy)

            # chunk 0: only piece pc0 runs up front (steps 0..TPP-1 need just
            # pc0); its later pieces interleave with the first steps below.
            # W and x transfers interleaved, small br/eyes constants last.
            pre = []
            defer0 = []
            emit_phase1(0, pre, defer0)
            for k in range(KB):
                w_load(k)
                pre[k]()                # x_k DMA
            emit_const_tail()
            for f in pre[KB:]:
                f()

            # ---- main loop ----
            for ch in range(NCH):
                par = ch % 2
                outc = OUTC[par]
                mbc = MBC[par]
                # deferred chunk-0 pieces + next chunk's phase-1, interleaved
                nxt = list(defer0) if ch == 0 else []
                if ch + 1 < NCH:
                    emit_phase1(ch + 1, nxt)
                nxt_per_step = -(-len(nxt) // TCH) if nxt else 0

                for tl in range(TCH):
                    t = ch * TCH + tl
                    # source column of mem state
                    if tl == 0:
                        mcol = MBC[(ch - 1) % 2] + TCH * 8 if ch > 0 else MBC[0]
                    else:
                        mcol = mbc + tl * 8
                    for q in range(2):
                        mg = mega[q]
                        s_cur = sg[q][t % 2]
                        ps = ps2p.tile([128, 8], f32, tag="u")
                        qc = q * 8
                        # u = proj[t] (+psum) ...
                        nc.tensor.matmul(
                            ps[:], eye1, proj[:, t * 16 + qc: t * 16 + qc + 8],
                            start=True, stop=False,
                        )
                        # ... - 0.1 * sum_o(mem)  (4 o-block reduce matmuls,
                        #     col (b2,k) read dup x4 via stride tricks)
                        for k in range(OB):
                            mv = mg[:, mcol + k: mcol + k + 5: 4]
                            mv = mv.rearrange("p (b x) -> p b x", x=1).broadcast_to((128, 2, 4))
                            nc.tensor.matmul(ps[:], ones_sb[:], mv,
                                             start=False, stop=False)
                        # ... + 0.9 * mem
                        nc.tensor.matmul(
                            ps[:], eye9, mg[:, mcol: mcol + 8],
                            start=False, stop=True,
                        )
                        # pages: 0 -> spike @ OUT slot tl, 1 -> mem' @ MB slot tl+1
                        outap = mg[:, outc + tl * 8: outc + tl * 8 + 8]
                        outap = outap.rearrange("p (x c) -> p x c", x=1).copy()
                        outap.ap[1] = (DPAGE, 2)
                        in0 = ps[:].rearrange("p (x c) -> p x c", x=1).broadcast_to((128, 2, 8))
                        in1 = s_cur[:].rearrange("p (x c) -> p x c", x=1).broadcast_to((128, 2, 8))
                        nc.vector._custom_dve(
                            PAIRFIRE, out=outap, in0=in0, in1=in1, s1=INHIB,
                        )
                    # sigma' = 0.9*sigma + spk (DVE stt: same-engine in-order
                    # after both pair ops -> zero sem cost, hides in latency)
                    for q in range(2):
                        spk = mega[q][:, outc + tl * 8: outc + tl * 8 + 8]
                        nc.vector.scalar_tensor_tensor(
                            sg[q][(t + 1) % 2][:], sg[q][t % 2][:], DECAY,
                            spk, Alu.mult, Alu.add,
                        )
                    # interleave next chunk's phase-1 work
                    for _ in range(nxt_per_step):
                        if nxt:
                            nxt.pop(0)()
                    # stream this chunk's spikes out early in pieces (the
                    # last chunk in quarters to shrink the end-of-kernel tail)
                    nq = 4 if ch == NCH - 1 else 2
                    if TCH >= nq and (tl + 1) % (TCH // nq) == 0 and tl + 1 < TCH:
                        piece = OUTW // nq
                        lo = ((tl + 1) // (TCH // nq) - 1) * piece
                        for q in range(2):
                            nc.scalar.dma_start(
                                out_d[:, q * n_steps * 8 + ch * OUTW + lo:
                                      q * n_steps * 8 + ch * OUTW + lo + piece],
                                mega[q][:, outc + lo:outc + lo + piece],
                            )
                for f in nxt:
                    f()
                # stream the rest of this chunk's spikes out
                nq = 4 if ch == NCH - 1 else 2
                h = OUTW - OUTW // nq if TCH >= nq else 0
                for q in range(2):
                    nc.scalar.dma_start(
                        out_d[:, q * n_steps * 8 + ch * OUTW + h:
                              q * n_steps * 8 + (ch + 1) * OUTW],
                        mega[q][:, outc + h:outc + OUTW],
                    )

    nc.compile()
    return nc


def _get_nc(n_steps):
    if n_steps not in _CACHE:
        _CACHE[n_steps] = _build(n_steps)
    return _CACHE[n_steps]


def _host_pack(x, W, b, n_steps):
    xs_all = []
    for c in range(8):
        xs = x[c * NB:(c + 1) * NB, :n_steps, :]          # [NB, S, I]
        xtc = np.ascontiguousarray(xs.transpose(2, 1, 0)).reshape(I, n_steps * NB)
        xs_all.append(xtc)
    wt = np.ascontiguousarray(W.T)                         # [I, O]
    wblk = np.concatenate([wt[k * 128:(k + 1) * 128] for k in range(4)], axis=1)
    br = np.ascontiguousarray(b.reshape(4, 128).T)         # [128, 4]
    eye = np.eye(128, dtype=np.float32)
    cst = np.concatenate(
        [wblk, br, eye, (np.float32(DECAY) * eye)], axis=1
    ).astype(np.float32)                                   # [128, 2308]
    return [{"xt": xs_all[c], "cst": cst} for c in range(8)]


def _host_unpack(outs, n_steps):
    full = np.empty((B, n_steps, O), dtype=np.float32)
    for c in range(8):
        o = outs[c]["out"].reshape(128, 2, n_steps, 2, 4)  # [o_in, q, t, b2, k]
        full[c * NB:(c + 1) * NB] = np.ascontiguousarray(
            o.transpose(1, 3, 2, 4, 0)).reshape(NB, n_steps, O)
    return full


def kernel(x, W, b, n_steps=S, trace=False):
    from concourse.bass_utils import run_bass_kernel_spmd

    x = np.asarray(x, dtype=np.float32)
    W = np.asarray(W, dtype=np.float32)
    b = np.asarray(b, dtype=np.float32)
    nc = _get_nc(n_steps)
    in_maps = _host_pack(x, W, b, n_steps)
    res = run_bass_kernel_spmd(nc, in_maps, core_ids=list(range(8)), trace=trace)
    out = _host_unpack(res.results, n_steps)
    kernel.last_result = res
    return out


# revision 37
# speedup vs baseline: 1.0075x; 1.0003x over previous
"""AdaptiveResonateAndFireNeuron Trainium2 kernel v2 (8 cores, data-parallel batch).

Per batch row b:
  proj[t] = x[b,t,:] @ W.T + bias
  u       = 0.9*mem + proj[t] - 0.1*sum_o(mem)
  spk     = (u >= thr);  mem' = u - spk*thr;  thr' = 0.9*thr + 0.1*spk

v2 structure (vs v1 baseline):
- phase 1 (proj) matmuls in float32r (1 cyc/row at 512 cols vs 4 for fp32);
  PSUM->SBUF copy + bias moved from DVE to the Activation engine.
- phase 2: u is accumulated ENTIRELY in PSUM by 6 matmuls per step per chain
  (identity@proj + 4x (-0.1*ones)@mem [o-block reduce] + 0.9*identity@mem).
  A single custom DVE op (ARF_PAIRFIRE, subdim pages) then reads PSUM once and
  writes BOTH outputs: page0 = spike (straight into the DMA staging buffer),
  page1 = mem' (into the mem trajectory ring).  Threshold state is kept scaled
  (sigma = 10*thr) and updated on the GpSimd/Pool engine with ONE
  scalar_tensor_tensor: sigma' = 0.9*sigma + spk.
- two independent chains (2 batches each) interleave so the PE->DVE->PE
  latency of one chain hides under the other.

Core layout: partition p = o_in (o % 128); per-step col c = b4*4 + k where
b4 = local batch (0..3), k = o_blk (o // 128); chain q = b4 // 2.
"""

import numpy as np

B, S, I, O = 32, 1024, 512, 512

SIGMA_ENGINE = "gpsimd"          # "gpsimd" (Pool) or "vector" (DVE)


def _SIGMA_ENG(nc):
    return getattr(nc, SIGMA_ENGINE)
NB = B // 8          # batches per core
DECAY = 0.9
INHIB = 0.1

_CACHE = {}


def _register_dve_ops():
    import concourse.dve_ops as dve_ops
    from concourse.dve_spec import Spec, Src0, Src1, C1, select, minn, SubIdx
    from concourse.dve_table_gen import dve_ver_for

    if "ARF_PAIRFIRE" in dve_ops._SUB_OPCODE_FOR_NAME:
        from concourse.dve_ops import OPS
        by_name = {op.name: op for op in OPS}
        return by_name["ARF_PAIRFIRE"]

    # in0 = u (PSUM, page-broadcast), in1 = sigma = 10*thr (page-broadcast),
    # s1 (C1) = 0.1.  h = 0.1*sigma = thr.
    # page 0 (SubIdx falsy) -> spike = (u >= thr)
    # page 1                -> mem'  = min(u, u - thr)   (valid: thr > 0)
    h = Src1 * C1
    body = select(SubIdx, minn(Src0, Src0 - h), Src0 >= h)

    def ref(in0, in1, s0, s1, imm2):
        u = in0.astype(np.float32)
        thr = (in1 * np.float32(s1)).astype(np.float32)
        spk = (u >= thr).astype(np.float32)
        memf = np.minimum(u, (u - thr).astype(np.float32))
        pages = np.arange(in0.shape[1])[None, :, None]
        return np.where(pages == 0, spk, memf).astype(np.float32)

    spec = Spec(body=body, reference=ref)
    op = dve_ops.DveOp("ARF_PAIRFIRE", spec, subdim=True, uops_sha={})
    dve_ops.OPS.append(op)
    dve_ops.CUSTOM_DVE_SPECS["ARF_PAIRFIRE"] = spec
    dve_ops._SUB_OPCODE_FOR_NAME["ARF_PAIRFIRE"] = (
        dve_ops._CUSTOM_DVE_ROW_BASE + len(dve_ops.OPS) - 1
    )
    ver = dve_ver_for("TRN2")
    try:
        op.compile(ver)
    except ValueError as e:
        import re

        m = re.search(r":\s*([0-9a-f]{8,})\s*≠", str(e))
        if m is None:
            raise
        op.uops_sha[ver] = m.group(1)
        dve_ops._COMPILE_CACHE.pop(("ARF_PAIRFIRE", ver), None)
        op.compile(ver)
    return op


def _build(n_steps):
    import concourse.bass as bass
    from concourse import bacc
    import concourse.mybir as mybir
    from concourse import tile

    PAIRFIRE = _register_dve_ops()

    f32 = mybir.dt.float32
    f32r = mybir.dt.float32r
    Alu = mybir.AluOpType
    Act = mybir.ActivationFunctionType

    nc = bacc.Bacc()
    # xt[i, t*4+b4] = x[b4, t, i]  (host pre-transposed, per core)
    xt = nc.declare_dram_parameter("xt", [I, NB * n_steps], f32, isOutput=False)
    # cst: [w blocks (4x512) | br (4) | eyes (256)] -- single constants DMA
    # cst[:, k*512+o] = W[o, k*128+p]; cst[:, 2048+k] = bias[k*128+p]
    # cst[:, 2052:2180] = I; cst[:, 2180:2308] = 0.9*I
    cst = nc.declare_dram_parameter("cst", [128, 2308], f32, isOutput=False)
    # out[o_in, q*(n_steps*8) + t*8 + b2*4 + k] = spikes[q*2+b2, t, k*128+o_in]
    out_d = nc.declare_dram_parameter("out", [128, n_steps * 16], f32, isOutput=True)

    KB = I // 128            # 4 contraction blocks
    OB = O // 128            # 4 output blocks
    TCH = min(128, n_steps)  # timesteps per chunk
    assert n_steps % TCH == 0
    NCH = n_steps // TCH

    # per-chain mega-tile column layout: [OUT0 | MB0 | OUT1 | MB1]
    # (separate tile per chain so hazard intervals never cross chains)
    OUTW = TCH * 8
    MBW = (TCH + 1) * 8
    OUTC = [0, OUTW + MBW]
    MBC = [OUTW, 2 * OUTW + MBW]
    MEGA_W = 2 * (OUTW + MBW)
    DPAGE = MBC[0] - OUTC[0] + 8      # +8: mem slot t+1 vs spike slot t

    with tile.TileContext(nc) as tc:
        with (
            tc.tile_pool(name="const", bufs=1) as constp,
            tc.tile_pool(name="state", bufs=1) as statep,
            tc.tile_pool(name="big", bufs=1) as bigp,
            tc.tile_pool(name="xin", bufs=4) as xinp,
            tc.tile_pool(name="ps1", bufs=4, space=bass.MemorySpace.PSUM) as ps1p,
            tc.tile_pool(name="ps2", bufs=4, space=bass.MemorySpace.PSUM) as ps2p,
        ):
            # ---- constants (one DMA + ACT proxy copies so PE waits on one sem) ----
            c_raw = constp.tile([128, 2308], f32, tag="craw")
            w_sb = constp.tile([128, KB * O], f32, tag="w")
            br_sb = constp.tile([128, 4], f32, tag="br")
            ey_sb = constp.tile([128, 256], f32, tag="ey")
            ones_sb = constp.tile([128, 128], f32, tag="ones")
            warm = constp.tile([128, 1], f32, tag="warm")
            # tiny ACT op up front: act-table load overlaps the DMAs
            nc.vector.memset(warm[:], 0.0)
            nc.scalar.activation(warm[:], warm[:], Act.Identity, bias=0.0, scale=1.0)
            # per-k W DMA + copy thunks; interleaved with the x DMAs below so
            # the k=0 matmuls start after just two transfers
            def w_load(k):
                nc.gpsimd.dma_start(c_raw[:, k * O:(k + 1) * O],
                                    cst[:, k * O:(k + 1) * O])
                nc.scalar.copy(w_sb[:, k * O:(k + 1) * O].bitcast(f32r),
                               c_raw[:, k * O:(k + 1) * O])

            def emit_const_tail():
                nc.gpsimd.dma_start(c_raw[:, KB * O:], cst[:, KB * O:])
                nc.scalar.copy(br_sb[:], c_raw[:, KB * O:KB * O + 4])
                nc.scalar.copy(ey_sb[:], c_raw[:, KB * O + 4:KB * O + 260])

            nc.vector.memset(ones_sb[:], -INHIB)
            eye1 = ey_sb[:, 0:128]
            eye9 = ey_sb[:, 128:256]



            # ---- persistent buffers ----
            proj = bigp.tile([128, n_steps * 16], f32, tag="proj")
            megaA = bigp.tile([128, MEGA_W], f32, tag="megaA")
            megaB = bigp.tile([128, MEGA_W], f32, tag="megaB")
            mega = [megaA, megaB]
            proj4 = proj[:].rearrange("p (t b o) -> p t b o", b=NB, o=OB)

            sgA0 = statep.tile([128, 8], f32, tag="sgA0")
            sgA1 = statep.tile([128, 8], f32, tag="sgA1")
            sgB0 = statep.tile([128, 8], f32, tag="sgB0")
            sgB1 = statep.tile([128, 8], f32, tag="sgB1")
            sg = [[sgA0, sgA1], [sgB0, sgB1]]
            s09 = statep.tile([128, 8], f32, tag="s09")   # Pool scratch: 0.9*sigma
            for q in range(2):
                nc.vector.memset(sg[q][0][:], 10.0)       # sigma = 10*thr, thr0=1
                nc.vector.memset(mega[q][:, MBC[0]:MBC[0] + 8], 0.0)   # mem0 = 0

            def emit_phase1(ch, sink, sink2=None):
                """Emit phase-1 work units for chunk ch into sink (list of
                thunks); caller interleaves them between steps.  If sink2 is
                given, pieces pc>=1 go there instead (lets chunk 0's later
                pieces interleave with its first steps)."""
                c0 = ch * TCH * NB
                xr = xinp.tile([128, KB * TCH * NB], f32, tag="xraw")
                xa = xinp.tile([128, KB * TCH * NB], f32, tag="x")
                xoff = [k * TCH * NB for k in range(KB)]
                for k in range(KB):
                    def dma(xr=xr, c0=c0, k=k):
                        nc.gpsimd.dma_start(
                            xr[:, xoff[k]:xoff[k] + TCH * NB],
                            xt[k * 128:(k + 1) * 128, c0:c0 + TCH * NB])
                    sink.append(dma)
                # 256-col matmul pieces so each fits a PE idle window between
                # recurrence steps without delaying them; pc-major order +
                # per-piece ACT copies so the chunk's first steps unblock
                # after ~1/NPC of the work
                PC = min(256, TCH * NB)
                NPC = (TCH * NB) // PC
                TPP = PC // NB           # timesteps covered per piece
                pss = []
                for ob in range(OB):
                    ps = ps1p.tile([128, TCH * NB], f32, tag="mmps")
                    pss.append(ps)
                for pc in range(NPC):
                    tgt = sink if (pc == 0 or sink2 is None) else sink2
                    # fp32r-rounding proxy copies for this piece's x columns
                    for k in range(KB):
                        def xcp(xr=xr, xa=xa, xo=xoff[k], pc=pc):
                            nc.scalar.copy(
                                xa[:, xo + pc * PC: xo + (pc + 1) * PC].bitcast(f32r),
                                xr[:, xo + pc * PC: xo + (pc + 1) * PC])
                        tgt.append(xcp)
                    for ob in range(OB):
                        ps = pss[ob]
                        for k in range(KB):
                            def mm(ps=ps, k=k, ob=ob, xo=xoff[k], xa=xa, pc=pc):
                                nc.tensor.matmul(
                                    ps[:, pc * PC:(pc + 1) * PC],
                                    w_sb[:, k * O + ob * 128: k * O + ob * 128 + 128].bitcast(f32r),
                                    xa[:, xo + pc * PC: xo + (pc + 1) * PC].bitcast(f32r),
                                    start=(k == 0),
                                    stop=(k == KB - 1),
                                )
                            tgt.append(mm)
                        def cpy(ps=ps, ob=ob, ch=ch, pc=pc):
                            src = ps[:, pc * PC:(pc + 1) * PC].rearrange(
                                "p (t b) -> p t b", b=NB)
                            dst = proj4[:, ch * TCH + pc * TPP:
                                        ch * TCH + (pc + 1) * TPP, :, ob]
                            nc.scalar.activation(
                                dst, src, Act.Identity,
                                bias=br_sb[:, ob:ob + 1], scale=1.0,
                            )
                        tgt.append(cpy)

            # chunk 0: only piece pc0 runs up front (steps 0..TPP-1 need just
            # pc0); its later pieces interleave with the first steps below.
            # W and x transfers interleaved, small br/eyes constants last.
            pre = []
            defer0 = []
            emit_phase1(0, pre, defer0)
            for k in range(KB):
                w_load(k)
                pre[k]()                # x_k DMA
            emit_const_tail()
            for f in pre[KB:]:
                f()

            # ---- main loop ----
            for ch in range(NCH):
                par = ch % 2
                outc = OUTC[par]
                mbc = MBC[par]
                # deferred chunk-0 pieces + next chunk's phase-1, interleaved
                nxt = list(defer0) if ch == 0 else []
                if ch + 1 < NCH:
                    emit_phase1(ch + 1, nxt)
                nxt_per_step = -(-len(nxt) // TCH) if nxt else 0

                for tl in range(TCH):
                    t = ch * TCH + tl
                    # source column of mem state
                    if tl == 0:
                        mcol = MBC[(ch - 1) % 2] + TCH * 8 if ch > 0 else MBC[0]
                    else:
                        mcol = mbc + tl * 8
                    for q in range(2):
                        mg = mega[q]
                        s_cur = sg[q][t % 2]
                        ps = ps2p.tile([128, 8], f32, tag="u")
                        qc = q * 8
                        # u = proj[t] (+psum) ...
                        nc.tensor.matmul(
                            ps[:], eye1, proj[:, t * 16 + qc: t * 16 + qc + 8],
                            start=True, stop=False,
                        )
                        # ... - 0.1 * sum_o(mem)  (4 o-block reduce matmuls,
                        #     col (b2,k) read dup x4 via stride tricks)
                        for k in range(OB):
                            mv = mg[:, mcol + k: mcol + k + 5: 4]
                            mv = mv.rearrange("p (b x) -> p b x", x=1).broadcast_to((128, 2, 4))
                            nc.tensor.matmul(ps[:], ones_sb[:], mv,
                                             start=False, stop=False)
                        # ... + 0.9 * mem
                        nc.tensor.matmul(
                            ps[:], eye9, mg[:, mcol: mcol + 8],
                            start=False, stop=True,
                        )
                        # pages: 0 -> spike @ OUT slot tl, 1 -> mem' @ MB slot tl+1
                        outap = mg[:, outc + tl * 8: outc + tl * 8 + 8]
                        outap = outap.rearrange("p (x c) -> p x c", x=1).copy()
                        outap.ap[1] = (DPAGE, 2)
                        in0 = ps[:].rearrange("p (x c) -> p x c", x=1).broadcast_to((128, 2, 8))
                        in1 = s_cur[:].rearrange("p (x c) -> p x c", x=1).broadcast_to((128, 2, 8))
                        nc.vector._custom_dve(
                            PAIRFIRE, out=outap, in0=in0, in1=in1, s1=INHIB,
                        )
                    # sigma' = 0.9*sigma + spk (DVE stt: same-engine in-order
                    # after both pair ops -> zero sem cost, hides in latency)
                    for q in range(2):
                        spk = mega[q][:, outc + tl * 8: outc + tl * 8 + 8]
                        nc.vector.scalar_tensor_tensor(
                            sg[q][(t + 1) % 2][:], sg[q][t % 2][:], DECAY,
                            spk, Alu.mult, Alu.add,
                        )
                    # interleave next chunk's phase-1 work
                    for _ in range(nxt_per_step):
                        if nxt:
                            nxt.pop(0)()
                    # stream this chunk's spikes out early in pieces (the
                    # last chunk in quarters to shrink the end-of-kernel tail)
                    nq = 8 if ch == NCH - 1 else 2
                    if TCH >= nq and (tl + 1) % (TCH // nq) == 0 and tl + 1 < TCH:
                        piece = OUTW // nq
                        lo = ((tl + 1) // (TCH // nq) - 1) * piece
                        for q in range(2):
                            nc.scalar.dma_start(
                                out_d[:, q * n_steps * 8 + ch * OUTW + lo:
                                      q * n_steps * 8 + ch * OUTW + lo + piece],
                                mega[q][:, outc + lo:outc + lo + piece],
                            )
                for f in nxt:
                    f()
                # stream the rest of this chunk's spikes out
                nq = 8 if ch == NCH - 1 else 2
                h = OUTW - OUTW // nq if TCH >= nq else 0
                for q in range(2):
                    nc.scalar.dma_start(
                        out_d[:, q * n_steps * 8 + ch * OUTW + h:
                              q * n_steps * 8 + (ch + 1) * OUTW],
                        mega[q][:, outc + h:outc + OUTW],
                    )

    nc.compile()
    return nc


def _get_nc(n_steps):
    if n_steps not in _CACHE:
        _CACHE[n_steps] = _build(n_steps)
    return _CACHE[n_steps]


def _host_pack(x, W, b, n_steps):
    xs_all = []
    for c in range(8):
        xs = x[c * NB:(c + 1) * NB, :n_steps, :]          # [NB, S, I]
        xtc = np.ascontiguousarray(xs.transpose(2, 1, 0)).reshape(I, n_steps * NB)
        xs_all.append(xtc)
    wt = np.ascontiguousarray(W.T)                         # [I, O]
    wblk = np.concatenate([wt[k * 128:(k + 1) * 128] for k in range(4)], axis=1)
    br = np.ascontiguousarray(b.reshape(4, 128).T)         # [128, 4]
    eye = np.eye(128, dtype=np.float32)
    cst = np.concatenate(
        [wblk, br, eye, (np.float32(DECAY) * eye)], axis=1
    ).astype(np.float32)                                   # [128, 2308]
    return [{"xt": xs_all[c], "cst": cst} for c in range(8)]


def _host_unpack(outs, n_steps):
    full = np.empty((B, n_steps, O), dtype=np.float32)
    for c in range(8):
        o = outs[c]["out"].reshape(128, 2, n_steps, 2, 4)  # [o_in, q, t, b2, k]
        full[c * NB:(c + 1) * NB] = np.ascontiguousarray(
            o.transpose(1, 3, 2, 4, 0)).reshape(NB, n_steps, O)
    return full


def kernel(x, W, b, n_steps=S, trace=False):
    from concourse.bass_utils import run_bass_kernel_spmd

    x = np.asarray(x, dtype=np.float32)
    W = np.asarray(W, dtype=np.float32)
    b = np.asarray(b, dtype=np.float32)
    nc = _get_nc(n_steps)
    in_maps = _host_pack(x, W, b, n_steps)
    res = run_bass_kernel_spmd(nc, in_maps, core_ids=list(range(8)), trace=trace)
    out = _host_unpack(res.results, n_steps)
    kernel.last_result = res
    return out
